# revision 5
# baseline (speedup 1.0000x reference)
"""Multi-head attention (B=2, S=2048, D=1024, H=16) on 8 Trainium2 cores.

Sharding: core = (batch b in {0,1}) x (head-group g in {0..3}, 4 heads each).
Each core computes its 4 heads end-to-end (Q/K/V projections restricted to the
group's 256 dims, attention, and the row-slice of the output projection) and
returns a partial [S, D] output; the host sums the 4 group partials per batch.

Device-side layouts (per core):
  qT/kT/vT  [D, S]   input activations, transposed on host, bf16
  wqT/wkT/wvT [D, 256] weight column-slices (wqT pre-scaled by 1/sqrt(dk)), bf16
  woT       [256, D] w_o.T row-slice, bf16
  bq/bk     [128, 2] per-partition bias ((b/8 for q), laid out [p, head-pair])
  out       [S, D]   fp32 partial output

b_v and b_o are handled exactly on the host: softmax rows sum to 1, so the
V-bias contributes b_v @ w_o.T + b_o as a constant row vector.
"""

import numpy as np

B, S, D = 2, 2048, 1024
H, DK = 16, 64
NCORES = 8
NGROUPS = 4                  # head-groups; 4 heads = 256 dims per group
GD = (H // NGROUPS) * DK     # 256 dims per group
NPAIR = 2                    # head-pairs per group (2 heads = 128 dims each)
SB = 512                     # s-block (matmul free dim / PSUM bank)
NSB = S // SB                # 4 s-blocks
NKT = S // 128               # 16 k-tiles of 128
NDT = D // 128               # 8 contraction tiles for projections
KG = 2                       # k-tiles per exp group (exp width = KG*512)

_CACHE = {}


def _build_program():
    from concourse import bacc, tile
    import concourse.mybir as mybir

    dt = mybir.dt
    nc = bacc.Bacc("TRN2", target_bir_lowering=False, debug=False,
                   num_devices=NCORES)

    qT = nc.dram_tensor("qT", [D, S], dt.bfloat16, kind="ExternalInput").ap()
    kT = nc.dram_tensor("kT", [D, S], dt.bfloat16, kind="ExternalInput").ap()
    vT = nc.dram_tensor("vT", [D, S], dt.bfloat16, kind="ExternalInput").ap()
    wqT = nc.dram_tensor("wqT", [D, GD], dt.bfloat16, kind="ExternalInput").ap()
    wkT = nc.dram_tensor("wkT", [D, GD], dt.bfloat16, kind="ExternalInput").ap()
    wvT = nc.dram_tensor("wvT", [D, GD], dt.bfloat16, kind="ExternalInput").ap()
    woT = nc.dram_tensor("woT", [GD, D], dt.bfloat16, kind="ExternalInput").ap()
    bq = nc.dram_tensor("bq", [128, NPAIR], dt.float32, kind="ExternalInput").ap()
    bk = nc.dram_tensor("bk", [128, NPAIR], dt.float32, kind="ExternalInput").ap()
    out = nc.dram_tensor("out", [S, D], dt.float32, kind="ExternalOutput").ap()

    qT_t = qT.rearrange("(t p) s -> t p s", p=128)   # [8, 128, S]
    kT_t = kT.rearrange("(t p) s -> t p s", p=128)
    vT_t = vT.rearrange("(t p) s -> t p s", p=128)
    wqT_t = wqT.rearrange("(t p) m -> t p m", p=128)  # [8, 128, GD]
    wkT_t = wkT.rearrange("(t p) m -> t p m", p=128)
    wvT_t = wvT.rearrange("(t p) m -> t p m", p=128)
    woT_t = woT.rearrange("(t p) m -> t p m", p=128)  # [2, 128, D]

    Exp = mybir.ActivationFunctionType.Exp

    with tile.TileContext(nc) as tc:
        with (
            tc.tile_pool(name="const", bufs=1) as const,
            tc.tile_pool(name="xin", bufs=12) as xin,
            tc.tile_pool(name="acts", bufs=1) as acts,
            tc.tile_pool(name="estage", bufs=4) as estage,
            tc.tile_pool(name="norm", bufs=4) as norm,
            tc.tile_pool(name="ostage", bufs=4) as ostage,
            tc.tile_pool(name="psum", bufs=1, space="PSUM") as psum,
        ):
            # ---- resident constants -------------------------------------
            wq_sb = const.tile([128, NDT, GD], dt.bfloat16, tag="wq")
            wk_sb = const.tile([128, NDT, GD], dt.bfloat16, tag="wk")
            wv_sb = const.tile([128, NDT, GD], dt.bfloat16, tag="wv")
            wo_sb = const.tile([128, NPAIR, D], dt.bfloat16, tag="wo")
            bq_sb = const.tile([128, NPAIR], dt.float32, tag="bq")
            bk_sb = const.tile([128, NPAIR], dt.float32, tag="bk")
            for t in range(NDT):
                nc.sync.dma_start(wq_sb[:, t, :], wqT_t[t])
                nc.sync.dma_start(wk_sb[:, t, :], wkT_t[t])
                nc.sync.dma_start(wv_sb[:, t, :], wvT_t[t])
            for t in range(NPAIR):
                nc.sync.dma_start(wo_sb[:, t, :], woT_t[t])
            nc.sync.dma_start(bq_sb[:], bq)
            nc.sync.dma_start(bk_sb[:], bk)

            # ---- activation tiles (whole-group residents) ---------------
            # QT/KT: [d-within-pair(128), pair, s]; V_aug: [k(128), k-tile,
            # head(4), dk+1] with col 64 = ones (softmax denominator trick).
            qt_sb = acts.tile([128, NPAIR, S], dt.bfloat16, tag="qt")
            kt_sb = acts.tile([128, NPAIR, S], dt.bfloat16, tag="kt")
            va_sb = acts.tile([128, NKT, 4, DK + 1], dt.bfloat16, tag="va")
            yt_sb = acts.tile([128, NPAIR, S], dt.bfloat16, tag="yt")

            nc.vector.memset(va_sb[:, :, :, DK:DK + 1], 1.0)

            # ---- K / V / Q projections ----------------------------------
            # K:  KT[d_g, s] = sum_t wkT[t].T @ kT[t]   (+ bk per-partition)
            for nb in range(NSB):
                xt = [xin.tile([128, SB], dt.bfloat16, tag="xt", name="xt")
                      for _ in range(NDT)]
                for t in range(NDT):
                    nc.sync.dma_start(xt[t], kT_t[t, :, nb * SB:(nb + 1) * SB])
                for hp in range(NPAIR):
                    ps = psum.tile([128, SB], dt.float32, tag="mm", bufs=2, name="ps")
                    for t in range(NDT):
                        nc.tensor.matmul(
                            ps, wk_sb[:, t, hp * 128:(hp + 1) * 128], xt[t],
                            start=(t == 0), stop=(t == NDT - 1))
                    nc.vector.tensor_scalar_add(
                        kt_sb[:, hp, nb * SB:(nb + 1) * SB], ps,
                        bk_sb[:, hp:hp + 1])

            # V: V[s, d_g] = sum_t vT[t][:, s-tile].T @ wvT[t]  (natural layout)
            for nb in range(NSB):
                xt = [xin.tile([128, SB], dt.bfloat16, tag="xt", name="xt")
                      for _ in range(NDT)]
                for t in range(NDT):
                    nc.sync.dma_start(xt[t], vT_t[t, :, nb * SB:(nb + 1) * SB])
                for sv in range(SB // 128):
                    st_i = nb * (SB // 128) + sv
                    ps = psum.tile([128, SB], dt.float32, tag="mm", bufs=2, name="ps")[:, 0:GD]
                    for t in range(NDT):
                        nc.tensor.matmul(
                            ps, xt[t][:, sv * 128:(sv + 1) * 128],
                            wv_sb[:, t, :],
                            start=(t == 0), stop=(t == NDT - 1))
                    nc.vector.tensor_copy(
                        va_sb[:, st_i, :, 0:DK],
                        ps.rearrange("p (h d) -> p h d", h=4))

            # Q (pre-scaled by 1/8 on host), then attention per q-block.
            for qb in range(NSB):
                xt = [xin.tile([128, SB], dt.bfloat16, tag="xt", name="xt")
                      for _ in range(NDT)]
                for t in range(NDT):
                    nc.sync.dma_start(xt[t], qT_t[t, :, qb * SB:(qb + 1) * SB])
                for hp in range(NPAIR):
                    ps = psum.tile([128, SB], dt.float32, tag="mm", bufs=2, name="ps")
                    for t in range(NDT):
                        nc.tensor.matmul(
                            ps, wq_sb[:, t, hp * 128:(hp + 1) * 128], xt[t],
                            start=(t == 0), stop=(t == NDT - 1))
                    nc.vector.tensor_scalar_add(
                        qt_sb[:, hp, qb * SB:(qb + 1) * SB], ps,
                        bq_sb[:, hp:hp + 1])

            # ---- attention ----------------------------------------------
            # Per (pair, q-block): scores^T tiles [k,q] for both heads of the
            # pair packed on disjoint PE row-groups; exp fused on ScalarE
            # (PSUM -> SBUF bf16) over KG k-tiles at once; PV accumulates
            # yT_aug[65, q] with lhsT = [V_h | 1] so row 64 = sum(exp).
            for qb in range(NSB):
                qsl = slice(qb * SB, (qb + 1) * SB)
                for hp in range(NPAIR):
                    ya = [psum.tile([DK + 1, SB], dt.float32, tag=f"y{j}", bufs=1,
                              name=f"y{j}") for j in range(2)]
                    for kg in range(NKT // KG):
                        stp = [psum.tile([128, KG, SB], dt.float32, tag=f"st{j}",
                                         bufs=1, name=f"st{j}") for j in range(2)]
                        for i in range(KG):
                            kk = kg * KG + i
                            ksl = slice(kk * 128, (kk + 1) * 128)
                            nc.tensor.matmul(
                                stp[0][:, i, :],
                                kt_sb[0:DK, hp, ksl], qt_sb[0:DK, hp, qsl],
                                start=True, stop=True, tile_position=(0, 0))
                            nc.tensor.matmul(
                                stp[1][:, i, :],
                                kt_sb[DK:128, hp, ksl], qt_sb[DK:128, hp, qsl],
                                start=True, stop=True, tile_position=(64, 0))
                        et = [estage.tile([128, KG, SB], dt.bfloat16,
                                          tag=f"e{j}", name=f"e{j}") for j in range(2)]
                        for j in range(2):
                            nc.scalar.activation(et[j][:], stp[j][:], Exp)
                        for i in range(KG):
                            kk = kg * KG + i
                            for j in range(2):
                                nc.tensor.matmul(
                                    ya[j], va_sb[:, kk, 2 * hp + j, :],
                                    et[j][:, i, :],
                                    start=(kg == 0 and i == 0),
                                    stop=(kg == NKT // KG - 1 and i == KG - 1))
                    # normalize: y[0:64] * (1/y[64]) broadcast along free dim
                    for j in range(2):
                        rr = norm.tile([1, SB], dt.float32, tag="rr")
                        nc.vector.reciprocal(rr, ya[j][DK:DK + 1, :])
                        rb = norm.tile([DK, SB], dt.float32, tag="rb")
                        nc.gpsimd.partition_broadcast(rb, rr)
                        nc.vector.tensor_mul(
                            yt_sb[j * DK:(j + 1) * DK, hp, qsl],
                            ya[j][0:DK, :], rb)

            # ---- output projection --------------------------------------
            # out[s, :] = sum_hp yT[:, hp, s-tile].T @ woT[hp]
            for st_i in range(S // 128):
                ssl = slice(st_i * 128, (st_i + 1) * 128)
                for nb in range(D // SB):
                    ps = psum.tile([128, SB], dt.float32, tag="mm", bufs=2, name="ps")
                    for hp in range(NPAIR):
                        nc.tensor.matmul(
                            ps, yt_sb[:, hp, ssl],
                            wo_sb[:, hp, nb * SB:(nb + 1) * SB],
                            start=(hp == 0), stop=(hp == NPAIR - 1))
                    ot = ostage.tile([128, SB], dt.float32, tag="ot")
                    nc.vector.tensor_copy(ot, ps)
                    nc.sync.dma_start(out[ssl, nb * SB:(nb + 1) * SB], ot)

    nc.compile()
    return nc


def _get_program():
    if "nc" not in _CACHE:
        _CACHE["nc"] = _build_program()
    return _CACHE["nc"]


def make_in_maps(q, k, v, w_q, b_q, w_k, b_k, w_v, b_v, w_o, b_o):
    import ml_dtypes
    bf16 = ml_dtypes.bfloat16
    scale = 1.0 / np.sqrt(np.float32(DK))

    wqT = np.ascontiguousarray(w_q.T * scale)
    wkT = np.ascontiguousarray(w_k.T)
    wvT = np.ascontiguousarray(w_v.T)
    woT = np.ascontiguousarray(w_o.T)

    in_maps = []
    for b in range(B):
        qT = np.ascontiguousarray(q[b].T).astype(bf16)
        kT = np.ascontiguousarray(k[b].T).astype(bf16)
        vT = np.ascontiguousarray(v[b].T).astype(bf16)
        for g in range(NGROUPS):
            sl = slice(g * GD, (g + 1) * GD)
            in_maps.append({
                "qT": qT, "kT": kT, "vT": vT,
                "wqT": np.ascontiguousarray(wqT[:, sl]).astype(bf16),
                "wkT": np.ascontiguousarray(wkT[:, sl]).astype(bf16),
                "wvT": np.ascontiguousarray(wvT[:, sl]).astype(bf16),
                "woT": np.ascontiguousarray(woT[sl, :]).astype(bf16),
                "bq": np.ascontiguousarray(
                    (b_q[sl] * scale).reshape(NPAIR, 128).T).astype(np.float32),
                "bk": np.ascontiguousarray(
                    b_k[sl].reshape(NPAIR, 128).T).astype(np.float32),
            })
    return in_maps


def gather(results, w_o, b_v, b_o):
    corr = (b_v.astype(np.float64) @ w_o.T.astype(np.float64)
            + b_o.astype(np.float64)).astype(np.float32)
    out = np.empty((B, S, D), np.float32)
    for b in range(B):
        acc = np.zeros((S, D), np.float64)
        for g in range(NGROUPS):
            acc += results[b * NGROUPS + g]["out"]
        out[b] = acc.astype(np.float32) + corr
    return out


def kernel(q, k, v, w_q, b_q, w_k, b_k, w_v, b_v, w_o, b_o):
    from concourse.bass_utils import run_bass_kernel_spmd

    nc = _get_program()
    in_maps = make_in_maps(q, k, v, w_q, b_q, w_k, b_k, w_v, b_v, w_o, b_o)
    res = run_bass_kernel_spmd(nc, in_maps, list(range(NCORES)))
    return gather(res.results, w_o, b_v, b_o)


# revision 6
# speedup vs baseline: 1.0113x; 1.0113x over previous
"""Multi-head attention (B=2, S=2048, D=1024, H=16) on 8 Trainium2 cores.

Sharding: core = (batch b in {0,1}) x (head-group g in {0..3}, 4 heads each).
Each core computes its 4 heads end-to-end (Q/K/V projections restricted to the
group's 256 dims, attention, and the row-slice of the output projection) and
returns a partial [S, D] output; the host sums the 4 group partials per batch.

Device-side layouts (per core):
  qT/kT/vT  [D, S]   input activations, transposed on host, bf16
  wqT/wkT/wvT [D, 256] weight column-slices (wqT pre-scaled by 1/sqrt(dk)), bf16
  woT       [256, D] w_o.T row-slice, bf16
  bq/bk     [128, 2] per-partition bias ((b/8 for q), laid out [p, head-pair])
  out       [S, D]   fp32 partial output

b_v and b_o are handled exactly on the host: softmax rows sum to 1, so the
V-bias contributes b_v @ w_o.T + b_o as a constant row vector.
"""

import numpy as np

B, S, D = 2, 2048, 1024
H, DK = 16, 64
NCORES = 8
NGROUPS = 4                  # head-groups; 4 heads = 256 dims per group
GD = (H // NGROUPS) * DK     # 256 dims per group
NPAIR = 2                    # head-pairs per group (2 heads = 128 dims each)
SB = 512                     # s-block (matmul free dim / PSUM bank)
NSB = S // SB                # 4 s-blocks
NKT = S // 128               # 16 k-tiles of 128
NDT = D // 128               # 8 contraction tiles for projections
KG = 2                       # k-tiles per exp group (exp width = KG*512)

_CACHE = {}


def _build_program():
    from concourse import bacc, tile
    import concourse.mybir as mybir

    dt = mybir.dt
    nc = bacc.Bacc("TRN2", target_bir_lowering=False, debug=False,
                   num_devices=NCORES)

    qT = nc.dram_tensor("qT", [D, S], dt.bfloat16, kind="ExternalInput").ap()
    kT = nc.dram_tensor("kT", [D, S], dt.bfloat16, kind="ExternalInput").ap()
    vT = nc.dram_tensor("vT", [D, S], dt.bfloat16, kind="ExternalInput").ap()
    wqT = nc.dram_tensor("wqT", [D, GD], dt.bfloat16, kind="ExternalInput").ap()
    wkT = nc.dram_tensor("wkT", [D, GD], dt.bfloat16, kind="ExternalInput").ap()
    wvT = nc.dram_tensor("wvT", [D, GD], dt.bfloat16, kind="ExternalInput").ap()
    woT = nc.dram_tensor("woT", [GD, D], dt.bfloat16, kind="ExternalInput").ap()
    bq = nc.dram_tensor("bq", [128, NPAIR], dt.float32, kind="ExternalInput").ap()
    bk = nc.dram_tensor("bk", [128, NPAIR], dt.float32, kind="ExternalInput").ap()
    out = nc.dram_tensor("out", [S, D], dt.float32, kind="ExternalOutput").ap()

    qT_t = qT.rearrange("(t p) s -> t p s", p=128)   # [8, 128, S]
    kT_t = kT.rearrange("(t p) s -> t p s", p=128)
    vT_t = vT.rearrange("(t p) s -> t p s", p=128)
    wqT_t = wqT.rearrange("(t p) m -> t p m", p=128)  # [8, 128, GD]
    wkT_t = wkT.rearrange("(t p) m -> t p m", p=128)
    wvT_t = wvT.rearrange("(t p) m -> t p m", p=128)
    woT_t = woT.rearrange("(t p) m -> t p m", p=128)  # [2, 128, D]

    Exp = mybir.ActivationFunctionType.Exp

    with tile.TileContext(nc) as tc:
        with (
            tc.tile_pool(name="const", bufs=1) as const,
            tc.tile_pool(name="xin", bufs=12) as xin,
            tc.tile_pool(name="acts", bufs=1) as acts,
            tc.tile_pool(name="estage", bufs=4) as estage,
            tc.tile_pool(name="norm", bufs=4) as norm,
            tc.tile_pool(name="ostage", bufs=4) as ostage,
            tc.tile_pool(name="psum", bufs=1, space="PSUM") as psum,
        ):
            # ---- resident constants -------------------------------------
            wq_sb = const.tile([128, NDT, GD], dt.bfloat16, tag="wq")
            wk_sb = const.tile([128, NDT, GD], dt.bfloat16, tag="wk")
            wv_sb = const.tile([128, NDT, GD], dt.bfloat16, tag="wv")
            wo_sb = const.tile([128, NPAIR, D], dt.bfloat16, tag="wo")
            bq_sb = const.tile([128, NPAIR], dt.float32, tag="bq")
            bk_sb = const.tile([128, NPAIR], dt.float32, tag="bk")
            for t in range(NDT):
                nc.sync.dma_start(wq_sb[:, t, :], wqT_t[t])
                nc.sync.dma_start(wk_sb[:, t, :], wkT_t[t])
                nc.sync.dma_start(wv_sb[:, t, :], wvT_t[t])
            for t in range(NPAIR):
                nc.sync.dma_start(wo_sb[:, t, :], woT_t[t])
            nc.sync.dma_start(bq_sb[:], bq)
            nc.sync.dma_start(bk_sb[:], bk)

            # ---- activation tiles (whole-group residents) ---------------
            # QT/KT: [d-within-pair(128), pair, s]; V_aug: [k(128), k-tile,
            # head(4), dk+1] with col 64 = ones (softmax denominator trick).
            qt_sb = acts.tile([128, NPAIR, S], dt.bfloat16, tag="qt")
            kt_sb = acts.tile([128, NPAIR, S], dt.bfloat16, tag="kt")
            va_sb = acts.tile([128, NKT, 4, DK + 1], dt.bfloat16, tag="va")
            yt_sb = acts.tile([128, NPAIR, S], dt.bfloat16, tag="yt")

            nc.vector.memset(va_sb[:, :, :, DK:DK + 1], 1.0)

            # ---- K / V / Q projections ----------------------------------
            # K:  KT[d_g, s] = sum_t wkT[t].T @ kT[t]   (+ bk per-partition)
            for nb in range(NSB):
                xt = [xin.tile([128, SB], dt.bfloat16, tag="xt", name="xt")
                      for _ in range(NDT)]
                for t in range(NDT):
                    nc.sync.dma_start(xt[t], kT_t[t, :, nb * SB:(nb + 1) * SB])
                for hp in range(NPAIR):
                    ps = psum.tile([128, SB], dt.float32, tag="mm", bufs=2, name="ps")
                    for t in range(NDT):
                        nc.tensor.matmul(
                            ps, wk_sb[:, t, hp * 128:(hp + 1) * 128], xt[t],
                            start=(t == 0), stop=(t == NDT - 1))
                    nc.vector.tensor_scalar_add(
                        kt_sb[:, hp, nb * SB:(nb + 1) * SB], ps,
                        bk_sb[:, hp:hp + 1])

            # V: V[s, d_g] = sum_t vT[t][:, s-tile].T @ wvT[t]  (natural layout)
            for nb in range(NSB):
                xt = [xin.tile([128, SB], dt.bfloat16, tag="xt", name="xt")
                      for _ in range(NDT)]
                for t in range(NDT):
                    nc.sync.dma_start(xt[t], vT_t[t, :, nb * SB:(nb + 1) * SB])
                for sv in range(SB // 128):
                    st_i = nb * (SB // 128) + sv
                    ps = psum.tile([128, SB], dt.float32, tag="mm", bufs=2, name="ps")[:, 0:GD]
                    for t in range(NDT):
                        nc.tensor.matmul(
                            ps, xt[t][:, sv * 128:(sv + 1) * 128],
                            wv_sb[:, t, :],
                            start=(t == 0), stop=(t == NDT - 1))
                    nc.vector.tensor_copy(
                        va_sb[:, st_i, :, 0:DK],
                        ps.rearrange("p (h d) -> p h d", h=4))

            # Q (pre-scaled by 1/8 on host), then attention per q-block.
            for qb in range(NSB):
                xt = [xin.tile([128, SB], dt.bfloat16, tag="xt", name="xt")
                      for _ in range(NDT)]
                for t in range(NDT):
                    nc.sync.dma_start(xt[t], qT_t[t, :, qb * SB:(qb + 1) * SB])
                for hp in range(NPAIR):
                    ps = psum.tile([128, SB], dt.float32, tag="mm", bufs=2, name="ps")
                    for t in range(NDT):
                        nc.tensor.matmul(
                            ps, wq_sb[:, t, hp * 128:(hp + 1) * 128], xt[t],
                            start=(t == 0), stop=(t == NDT - 1))
                    nc.vector.tensor_scalar_add(
                        qt_sb[:, hp, qb * SB:(qb + 1) * SB], ps,
                        bq_sb[:, hp:hp + 1])

            # ---- attention ----------------------------------------------
            # Per (pair, q-block): scores^T tiles [k,q] for both heads of the
            # pair packed on disjoint PE row-groups; exp fused on ScalarE
            # (PSUM -> SBUF bf16) over KG k-tiles at once; PV accumulates
            # yT_aug[65, q] with lhsT = [V_h | 1] so row 64 = sum(exp).
            for qb in range(NSB):
                qsl = slice(qb * SB, (qb + 1) * SB)
                for hp in range(NPAIR):
                    ya = psum.tile([DK + 1, 2, SB], dt.float32, tag="ya",
                                   bufs=1, name="ya")
                    for kk in range(NKT):
                        ksl = slice(kk * 128, (kk + 1) * 128)
                        st = psum.tile([128, 2, SB], dt.float32, tag="st",
                                       bufs=2, name="st")
                        nc.tensor.matmul(
                            st[:, 0, :],
                            kt_sb[0:DK, hp, ksl], qt_sb[0:DK, hp, qsl],
                            start=True, stop=True, tile_position=(0, 0))
                        nc.tensor.matmul(
                            st[:, 1, :],
                            kt_sb[DK:128, hp, ksl], qt_sb[DK:128, hp, qsl],
                            start=True, stop=True, tile_position=(64, 0))
                        et = estage.tile([128, 2, SB], dt.bfloat16, tag="et",
                                         name="et")
                        nc.scalar.activation(et[:], st[:], Exp)
                        for j in range(2):
                            nc.tensor.matmul(
                                ya[:, j, :], va_sb[:, kk, 2 * hp + j, :],
                                et[:, j, :],
                                start=(kk == 0), stop=(kk == NKT - 1))
                    # normalize: y[0:64] * (1/y[64]) broadcast along free dim
                    for j in range(2):
                        rr = norm.tile([1, SB], dt.float32, tag="rr")
                        nc.vector.reciprocal(rr, ya[DK:DK + 1, j, :])
                        rb = norm.tile([DK, SB], dt.float32, tag="rb")
                        nc.gpsimd.partition_broadcast(rb, rr)
                        nc.vector.tensor_mul(
                            yt_sb[j * DK:(j + 1) * DK, hp, qsl],
                            ya[0:DK, j, :], rb)

            # ---- output projection --------------------------------------
            # out[s, :] = sum_hp yT[:, hp, s-tile].T @ woT[hp]
            for st_i in range(S // 128):
                ssl = slice(st_i * 128, (st_i + 1) * 128)
                for nb in range(D // SB):
                    ps = psum.tile([128, SB], dt.float32, tag="mm", bufs=2, name="ps")
                    for hp in range(NPAIR):
                        nc.tensor.matmul(
                            ps, yt_sb[:, hp, ssl],
                            wo_sb[:, hp, nb * SB:(nb + 1) * SB],
                            start=(hp == 0), stop=(hp == NPAIR - 1))
                    ot = ostage.tile([128, SB], dt.float32, tag="ot")
                    nc.vector.tensor_copy(ot, ps)
                    nc.sync.dma_start(out[ssl, nb * SB:(nb + 1) * SB], ot)

    nc.compile()
    return nc


def _get_program():
    if "nc" not in _CACHE:
        _CACHE["nc"] = _build_program()
    return _CACHE["nc"]


def make_in_maps(q, k, v, w_q, b_q, w_k, b_k, w_v, b_v, w_o, b_o):
    import ml_dtypes
    bf16 = ml_dtypes.bfloat16
    scale = 1.0 / np.sqrt(np.float32(DK))

    wqT = np.ascontiguousarray(w_q.T * scale)
    wkT = np.ascontiguousarray(w_k.T)
    wvT = np.ascontiguousarray(w_v.T)
    woT = np.ascontiguousarray(w_o.T)

    in_maps = []
    for b in range(B):
        qT = np.ascontiguousarray(q[b].T).astype(bf16)
        kT = np.ascontiguousarray(k[b].T).astype(bf16)
        vT = np.ascontiguousarray(v[b].T).astype(bf16)
        for g in range(NGROUPS):
            sl = slice(g * GD, (g + 1) * GD)
            in_maps.append({
                "qT": qT, "kT": kT, "vT": vT,
                "wqT": np.ascontiguousarray(wqT[:, sl]).astype(bf16),
                "wkT": np.ascontiguousarray(wkT[:, sl]).astype(bf16),
                "wvT": np.ascontiguousarray(wvT[:, sl]).astype(bf16),
                "woT": np.ascontiguousarray(woT[sl, :]).astype(bf16),
                "bq": np.ascontiguousarray(
                    (b_q[sl] * scale).reshape(NPAIR, 128).T).astype(np.float32),
                "bk": np.ascontiguousarray(
                    b_k[sl].reshape(NPAIR, 128).T).astype(np.float32),
            })
    return in_maps


def gather(results, w_o, b_v, b_o):
    corr = (b_v.astype(np.float64) @ w_o.T.astype(np.float64)
            + b_o.astype(np.float64)).astype(np.float32)
    out = np.empty((B, S, D), np.float32)
    for b in range(B):
        acc = np.zeros((S, D), np.float64)
        for g in range(NGROUPS):
            acc += results[b * NGROUPS + g]["out"]
        out[b] = acc.astype(np.float32) + corr
    return out


def kernel(q, k, v, w_q, b_q, w_k, b_k, w_v, b_v, w_o, b_o):
    from concourse.bass_utils import run_bass_kernel_spmd

    nc = _get_program()
    in_maps = make_in_maps(q, k, v, w_q, b_q, w_k, b_k, w_v, b_v, w_o, b_o)
    res = run_bass_kernel_spmd(nc, in_maps, list(range(NCORES)))
    return gather(res.results, w_o, b_v, b_o)


# revision 7
# speedup vs baseline: 1.0314x; 1.0198x over previous
"""Multi-head attention (B=2, S=2048, D=1024, H=16) on 8 Trainium2 cores.

Sharding: core = (batch b in {0,1}) x (head-group g in {0..3}, 4 heads each).
Each core computes its 4 heads end-to-end (Q/K/V projections restricted to the
group's 256 dims, attention, and the row-slice of the output projection) and
returns a partial [S, D] output; the host sums the 4 group partials per batch.

Device-side layouts (per core):
  qT/kT/vT  [D, S]   input activations, transposed on host, bf16
  wqT/wkT/wvT [D, 256] weight column-slices (wqT pre-scaled by 1/sqrt(dk)), bf16
  woT       [256, D] w_o.T row-slice, bf16
  bq/bk     [128, 2] per-partition bias ((b/8 for q), laid out [p, head-pair])
  out       [S, D]   fp32 partial output

b_v and b_o are handled exactly on the host: softmax rows sum to 1, so the
V-bias contributes b_v @ w_o.T + b_o as a constant row vector.
"""

import numpy as np

B, S, D = 2, 2048, 1024
H, DK = 16, 64
NCORES = 8
NGROUPS = 4                  # head-groups; 4 heads = 256 dims per group
GD = (H // NGROUPS) * DK     # 256 dims per group
NPAIR = 2                    # head-pairs per group (2 heads = 128 dims each)
SB = 512                     # s-block (matmul free dim / PSUM bank)
NSB = S // SB                # 4 s-blocks
NKT = S // 128               # 16 k-tiles of 128
NDT = D // 128               # 8 contraction tiles for projections
KG = 2                       # k-tiles per exp group (exp width = KG*512)

_CACHE = {}


def _build_program():
    from concourse import bacc, tile
    import concourse.mybir as mybir

    dt = mybir.dt
    nc = bacc.Bacc("TRN2", target_bir_lowering=False, debug=False,
                   num_devices=NCORES)

    qT = nc.dram_tensor("qT", [D, S], dt.bfloat16, kind="ExternalInput").ap()
    kT = nc.dram_tensor("kT", [D, S], dt.bfloat16, kind="ExternalInput").ap()
    vT = nc.dram_tensor("vT", [D, S], dt.bfloat16, kind="ExternalInput").ap()
    wqT = nc.dram_tensor("wqT", [D, GD], dt.bfloat16, kind="ExternalInput").ap()
    wkT = nc.dram_tensor("wkT", [D, GD], dt.bfloat16, kind="ExternalInput").ap()
    wvT = nc.dram_tensor("wvT", [D, GD], dt.bfloat16, kind="ExternalInput").ap()
    woT = nc.dram_tensor("woT", [GD, D], dt.bfloat16, kind="ExternalInput").ap()
    bq = nc.dram_tensor("bq", [128, NPAIR], dt.float32, kind="ExternalInput").ap()
    bk = nc.dram_tensor("bk", [128, NPAIR], dt.float32, kind="ExternalInput").ap()
    out = nc.dram_tensor("out", [S, D], dt.float32, kind="ExternalOutput").ap()

    qT_t = qT.rearrange("(t p) s -> t p s", p=128)   # [8, 128, S]
    kT_t = kT.rearrange("(t p) s -> t p s", p=128)
    vT_t = vT.rearrange("(t p) s -> t p s", p=128)
    wqT_t = wqT.rearrange("(t p) m -> t p m", p=128)  # [8, 128, GD]
    wkT_t = wkT.rearrange("(t p) m -> t p m", p=128)
    wvT_t = wvT.rearrange("(t p) m -> t p m", p=128)
    woT_t = woT.rearrange("(t p) m -> t p m", p=128)  # [2, 128, D]

    Exp = mybir.ActivationFunctionType.Exp

    with tile.TileContext(nc) as tc:
        with (
            tc.tile_pool(name="const", bufs=1) as const,
            tc.tile_pool(name="xin", bufs=2) as xin,
            tc.tile_pool(name="acts", bufs=1) as acts,
            tc.tile_pool(name="estage", bufs=6) as estage,
            tc.tile_pool(name="norm", bufs=4) as norm,
            tc.tile_pool(name="ostage", bufs=3) as ostage,
            tc.tile_pool(name="psum", bufs=1, space="PSUM") as psum,
        ):
            # ---- resident constants -------------------------------------
            wq_sb = const.tile([128, NDT, GD], dt.bfloat16, tag="wq")
            wk_sb = const.tile([128, NDT, GD], dt.bfloat16, tag="wk")
            wv_sb = const.tile([128, NDT, GD], dt.bfloat16, tag="wv")
            wo_sb = const.tile([128, NPAIR, D], dt.bfloat16, tag="wo")
            bq_sb = const.tile([128, NPAIR], dt.float32, tag="bq")
            bk_sb = const.tile([128, NPAIR], dt.float32, tag="bk")
            nc.sync.dma_start(wq_sb[:], wqT.rearrange("(t p) m -> p t m", p=128))
            nc.sync.dma_start(wk_sb[:], wkT.rearrange("(t p) m -> p t m", p=128))
            nc.sync.dma_start(wv_sb[:], wvT.rearrange("(t p) m -> p t m", p=128))
            nc.sync.dma_start(wo_sb[:], woT.rearrange("(t p) m -> p t m", p=128))
            nc.sync.dma_start(bq_sb[:], bq)
            nc.sync.dma_start(bk_sb[:], bk)

            # ---- activation tiles (whole-group residents) ---------------
            # QT/KT: [d-within-pair(128), pair, s]; V_aug: [k(128), k-tile,
            # head(4), dk+1] with col 64 = ones (softmax denominator trick).
            qt_sb = acts.tile([128, NPAIR, S], dt.bfloat16, tag="qt")
            kt_sb = acts.tile([128, NPAIR, S], dt.bfloat16, tag="kt")
            va_sb = acts.tile([128, NKT, 4, DK + 1], dt.bfloat16, tag="va")
            yt_sb = acts.tile([128, NPAIR, S], dt.bfloat16, tag="yt")

            nc.vector.memset(va_sb[:, :, :, DK:DK + 1], 1.0)

            # ---- K / V projections, interleaved per s-block -------------
            # K:  KT[d_g, s] = sum_t wkT[t].T @ kT[t]   (+ bk per-partition)
            # V:  V[s, d_g]  = sum_t vT[t][:, s-tile].T @ wvT[t]
            kTp = kT.rearrange("(t p) s -> p t s", p=128)   # [128, 8, S]
            vTp = vT.rearrange("(t p) s -> p t s", p=128)
            qTp = qT.rearrange("(t p) s -> p t s", p=128)
            for nb in range(NSB):
                nsl = slice(nb * SB, (nb + 1) * SB)
                xk = xin.tile([128, NDT, SB], dt.bfloat16, tag="xk", name="xk")
                nc.sync.dma_start(xk, kTp[:, :, nsl])
                xv = xin.tile([128, NDT, SB], dt.bfloat16, tag="xv", name="xv")
                nc.sync.dma_start(xv, vTp[:, :, nsl])
                for hp in range(NPAIR):
                    ps = psum.tile([128, SB], dt.float32, tag="mm", bufs=2,
                                   name="ps")
                    for t in range(NDT):
                        nc.tensor.matmul(
                            ps, wk_sb[:, t, hp * 128:(hp + 1) * 128],
                            xk[:, t, :],
                            start=(t == 0), stop=(t == NDT - 1))
                    nc.vector.tensor_scalar_add(
                        kt_sb[:, hp, nsl], ps, bk_sb[:, hp:hp + 1])
                for sv in range(SB // 128):
                    st_i = nb * (SB // 128) + sv
                    ps = psum.tile([128, SB], dt.float32, tag="mm", bufs=2,
                                   name="ps")[:, 0:GD]
                    for t in range(NDT):
                        nc.tensor.matmul(
                            ps, xv[:, t, sv * 128:(sv + 1) * 128],
                            wv_sb[:, t, :],
                            start=(t == 0), stop=(t == NDT - 1))
                    nc.vector.tensor_copy(
                        va_sb[:, st_i, :, 0:DK],
                        ps.rearrange("p (h d) -> p h d", h=4))

            # ---- per q-block: Q projection -> attention -> out proj -----
            for qb in range(NSB):
                qsl = slice(qb * SB, (qb + 1) * SB)
                xq = xin.tile([128, NDT, SB], dt.bfloat16, tag="xq", name="xq")
                nc.sync.dma_start(xq, qTp[:, :, qsl])
                for hp in range(NPAIR):
                    ps = psum.tile([128, SB], dt.float32, tag="mm", bufs=2,
                                   name="ps")
                    for t in range(NDT):
                        nc.tensor.matmul(
                            ps, wq_sb[:, t, hp * 128:(hp + 1) * 128],
                            xq[:, t, :],
                            start=(t == 0), stop=(t == NDT - 1))
                    nc.vector.tensor_scalar_add(
                        qt_sb[:, hp, qsl], ps, bq_sb[:, hp:hp + 1])

                # attention: scores^T [k, q] for both heads of the pair on
                # disjoint PE row-groups into one 2-bank PSUM tile; one wide
                # exp (PSUM -> SBUF bf16); PV accumulates yT_aug[65, q] with
                # lhsT = [V_h | 1] so row 64 = sum(exp).
                for hp in range(NPAIR):
                    ya = psum.tile([DK + 1, 2, SB], dt.float32, tag="ya",
                                   bufs=1, name="ya")
                    for kk in range(NKT):
                        ksl = slice(kk * 128, (kk + 1) * 128)
                        st = psum.tile([128, 2, SB], dt.float32, tag="st",
                                       bufs=2, name="st")
                        nc.tensor.matmul(
                            st[:, 0, :],
                            kt_sb[0:DK, hp, ksl], qt_sb[0:DK, hp, qsl],
                            start=True, stop=True, tile_position=(0, 0))
                        nc.tensor.matmul(
                            st[:, 1, :],
                            kt_sb[DK:128, hp, ksl], qt_sb[DK:128, hp, qsl],
                            start=True, stop=True, tile_position=(64, 0))
                        et = estage.tile([128, 2, SB], dt.bfloat16, tag="et",
                                         name="et")
                        nc.scalar.activation(et[:], st[:], Exp)
                        for j in range(2):
                            nc.tensor.matmul(
                                ya[:, j, :], va_sb[:, kk, 2 * hp + j, :],
                                et[:, j, :],
                                start=(kk == 0), stop=(kk == NKT - 1))
                    # normalize: y[0:64] * (1/y[64]) broadcast along free dim
                    for j in range(2):
                        rr = norm.tile([1, SB], dt.float32, tag="rr")
                        nc.vector.reciprocal(rr, ya[DK:DK + 1, j, :])
                        rb = norm.tile([DK, SB], dt.float32, tag="rb")
                        nc.gpsimd.partition_broadcast(rb, rr)
                        nc.vector.tensor_mul(
                            yt_sb[j * DK:(j + 1) * DK, hp, qsl],
                            ya[0:DK, j, :], rb)

                # out[s, :] = sum_hp yT[:, hp, s-tile].T @ woT[hp]
                for sv in range(SB // 128):
                    st_i = qb * (SB // 128) + sv
                    ssl = slice(st_i * 128, (st_i + 1) * 128)
                    ot = ostage.tile([128, D], dt.float32, tag="ot", name="ot")
                    for nb2 in range(D // SB):
                        ps = psum.tile([128, SB], dt.float32, tag="mm", bufs=2,
                                       name="ps")
                        for hp in range(NPAIR):
                            nc.tensor.matmul(
                                ps, yt_sb[:, hp, ssl],
                                wo_sb[:, hp, nb2 * SB:(nb2 + 1) * SB],
                                start=(hp == 0), stop=(hp == NPAIR - 1))
                        nc.vector.tensor_copy(
                            ot[:, nb2 * SB:(nb2 + 1) * SB], ps)
                    nc.sync.dma_start(out[ssl, :], ot)

    nc.compile()
    return nc


def _get_program():
    if "nc" not in _CACHE:
        _CACHE["nc"] = _build_program()
    return _CACHE["nc"]


def make_in_maps(q, k, v, w_q, b_q, w_k, b_k, w_v, b_v, w_o, b_o):
    import ml_dtypes
    bf16 = ml_dtypes.bfloat16
    scale = 1.0 / np.sqrt(np.float32(DK))

    wqT = np.ascontiguousarray(w_q.T * scale)
    wkT = np.ascontiguousarray(w_k.T)
    wvT = np.ascontiguousarray(w_v.T)
    woT = np.ascontiguousarray(w_o.T)

    in_maps = []
    for b in range(B):
        qT = np.ascontiguousarray(q[b].T).astype(bf16)
        kT = np.ascontiguousarray(k[b].T).astype(bf16)
        vT = np.ascontiguousarray(v[b].T).astype(bf16)
        for g in range(NGROUPS):
            sl = slice(g * GD, (g + 1) * GD)
            in_maps.append({
                "qT": qT, "kT": kT, "vT": vT,
                "wqT": np.ascontiguousarray(wqT[:, sl]).astype(bf16),
                "wkT": np.ascontiguousarray(wkT[:, sl]).astype(bf16),
                "wvT": np.ascontiguousarray(wvT[:, sl]).astype(bf16),
                "woT": np.ascontiguousarray(woT[sl, :]).astype(bf16),
                "bq": np.ascontiguousarray(
                    (b_q[sl] * scale).reshape(NPAIR, 128).T).astype(np.float32),
                "bk": np.ascontiguousarray(
                    b_k[sl].reshape(NPAIR, 128).T).astype(np.float32),
            })
    return in_maps


def gather(results, w_o, b_v, b_o):
    corr = (b_v.astype(np.float64) @ w_o.T.astype(np.float64)
            + b_o.astype(np.float64)).astype(np.float32)
    out = np.empty((B, S, D), np.float32)
    for b in range(B):
        acc = np.zeros((S, D), np.float64)
        for g in range(NGROUPS):
            acc += results[b * NGROUPS + g]["out"]
        out[b] = acc.astype(np.float32) + corr
    return out


def kernel(q, k, v, w_q, b_q, w_k, b_k, w_v, b_v, w_o, b_o):
    from concourse.bass_utils import run_bass_kernel_spmd

    nc = _get_program()
    in_maps = make_in_maps(q, k, v, w_q, b_q, w_k, b_k, w_v, b_v, w_o, b_o)
    res = run_bass_kernel_spmd(nc, in_maps, list(range(NCORES)))
    return gather(res.results, w_o, b_v, b_o)


# revision 9
# speedup vs baseline: 1.1021x; 1.0686x over previous
"""Multi-head attention (B=2, S=2048, D=1024, H=16) on 8 Trainium2 cores.

Sharding: core = (batch b in {0,1}) x (head-group g in {0..3}, 4 heads each).
Each core computes its 4 heads end-to-end (Q/K/V projections restricted to the
group's 256 dims, attention, and the row-slice of the output projection) and
returns a partial [S, D] output; the host sums the 4 group partials per batch.

Device-side layouts (per core):
  qT/kT/vT  [D, S]   input activations, transposed on host, bf16
  wqT/wkT/wvT [D, 256] weight column-slices (wqT pre-scaled by 1/sqrt(dk)), bf16
  woT       [256, D] w_o.T row-slice, bf16
  bq/bk     [128, 2] per-partition bias ((b/8 for q), laid out [p, head-pair])
  out       [S, D]   fp32 partial output

b_v and b_o are handled exactly on the host: softmax rows sum to 1, so the
V-bias contributes b_v @ w_o.T + b_o as a constant row vector.
"""

import numpy as np

B, S, D = 2, 2048, 1024
H, DK = 16, 64
NCORES = 8
NGROUPS = 4                  # head-groups; 4 heads = 256 dims per group
GD = (H // NGROUPS) * DK     # 256 dims per group
NPAIR = 2                    # head-pairs per group (2 heads = 128 dims each)
SB = 512                     # s-block (matmul free dim / PSUM bank)
NSB = S // SB                # 4 s-blocks
NKT = S // 128               # 16 k-tiles of 128
NDT = D // 128               # 8 contraction tiles for projections
KG = 2                       # k-tiles per exp group (exp width = KG*512)

_CACHE = {}


def _build_program():
    from concourse import bacc, tile
    import concourse.mybir as mybir

    dt = mybir.dt
    nc = bacc.Bacc("TRN2", target_bir_lowering=False, debug=False,
                   num_devices=NCORES)

    qT = nc.dram_tensor("qT", [D, S], dt.bfloat16, kind="ExternalInput").ap()
    kT = nc.dram_tensor("kT", [D, S], dt.bfloat16, kind="ExternalInput").ap()
    vT = nc.dram_tensor("vT", [D, S], dt.bfloat16, kind="ExternalInput").ap()
    wqT = nc.dram_tensor("wqT", [D, GD], dt.bfloat16, kind="ExternalInput").ap()
    wkT = nc.dram_tensor("wkT", [D, GD], dt.bfloat16, kind="ExternalInput").ap()
    wvT = nc.dram_tensor("wvT", [D, GD], dt.bfloat16, kind="ExternalInput").ap()
    woT = nc.dram_tensor("woT", [GD, D], dt.bfloat16, kind="ExternalInput").ap()
    bq = nc.dram_tensor("bq", [128, NPAIR], dt.float32, kind="ExternalInput").ap()
    bk = nc.dram_tensor("bk", [128, NPAIR], dt.float32, kind="ExternalInput").ap()
    out = nc.dram_tensor("out", [S, D], dt.float32, kind="ExternalOutput").ap()

    qT_t = qT.rearrange("(t p) s -> t p s", p=128)   # [8, 128, S]
    kT_t = kT.rearrange("(t p) s -> t p s", p=128)
    vT_t = vT.rearrange("(t p) s -> t p s", p=128)
    wqT_t = wqT.rearrange("(t p) m -> t p m", p=128)  # [8, 128, GD]
    wkT_t = wkT.rearrange("(t p) m -> t p m", p=128)
    wvT_t = wvT.rearrange("(t p) m -> t p m", p=128)
    woT_t = woT.rearrange("(t p) m -> t p m", p=128)  # [2, 128, D]

    Exp = mybir.ActivationFunctionType.Exp
    Log = mybir.ActivationFunctionType.Ln

    with tile.TileContext(nc) as tc:
        with (
            tc.tile_pool(name="const", bufs=1) as const,
            tc.tile_pool(name="xin", bufs=2) as xin,
            tc.tile_pool(name="acts", bufs=1) as acts,
            tc.tile_pool(name="estage", bufs=8) as estage,
            tc.tile_pool(name="norm", bufs=4) as norm,
            tc.tile_pool(name="ostage", bufs=3) as ostage,
            tc.tile_pool(name="psum", bufs=1, space="PSUM") as psum,
        ):
            # ---- resident constants -------------------------------------
            wq_sb = const.tile([128, NDT, GD], dt.bfloat16, tag="wq")
            wk_sb = const.tile([128, NDT, GD], dt.bfloat16, tag="wk")
            wv_sb = const.tile([128, NDT, GD], dt.bfloat16, tag="wv")
            wo_sb = const.tile([128, NPAIR, D], dt.bfloat16, tag="wo")
            bq_sb = const.tile([128, NPAIR], dt.float32, tag="bq")
            bk_sb = const.tile([128, NPAIR], dt.float32, tag="bk")
            nc.sync.dma_start(wq_sb[:], wqT.rearrange("(t p) m -> p t m", p=128))
            nc.sync.dma_start(wk_sb[:], wkT.rearrange("(t p) m -> p t m", p=128))
            nc.sync.dma_start(wv_sb[:], wvT.rearrange("(t p) m -> p t m", p=128))
            nc.sync.dma_start(wo_sb[:], woT.rearrange("(t p) m -> p t m", p=128))
            nc.sync.dma_start(bq_sb[:], bq)
            nc.sync.dma_start(bk_sb[:], bk)

            # ---- activation tiles (whole-group residents) ---------------
            # QT/KT: [d-within-pair(128), pair, s]; V_aug: [k(128), k-tile,
            # head(4), dk+1] with col 64 = ones (softmax denominator trick).
            qt_sb = acts.tile([128, NPAIR, S], dt.bfloat16, tag="qt")
            kt_sb = acts.tile([128, NPAIR, S], dt.bfloat16, tag="kt")
            va_sb = acts.tile([128, NKT, 4, DK + 1], dt.bfloat16, tag="va")
            yt_sb = acts.tile([128, NPAIR, S], dt.bfloat16, tag="yt")

            nc.vector.memset(va_sb[:, :, :, DK:DK + 1], 1.0)

            # ---- K / V projections, interleaved per s-block -------------
            # K:  KT[d_g, s] = sum_t wkT[t].T @ kT[t]   (+ bk per-partition)
            # V:  V[s, d_g]  = sum_t vT[t][:, s-tile].T @ wvT[t]
            kTp = kT.rearrange("(t p) s -> p t s", p=128)   # [128, 8, S]
            vTp = vT.rearrange("(t p) s -> p t s", p=128)
            qTp = qT.rearrange("(t p) s -> p t s", p=128)
            for nb in range(NSB):
                nsl = slice(nb * SB, (nb + 1) * SB)
                xk = xin.tile([128, NDT, SB], dt.bfloat16, tag="xk", name="xk")
                nc.sync.dma_start(xk, kTp[:, :, nsl])
                xv = xin.tile([128, NDT, SB], dt.bfloat16, tag="xv", name="xv")
                nc.sync.dma_start(xv, vTp[:, :, nsl])
                for hp in range(NPAIR):
                    ps = psum.tile([128, SB], dt.float32, tag="mm", bufs=2,
                                   name="ps")
                    for t in range(NDT):
                        nc.tensor.matmul(
                            ps, wk_sb[:, t, hp * 128:(hp + 1) * 128],
                            xk[:, t, :],
                            start=(t == 0), stop=(t == NDT - 1))
                    nc.vector.tensor_scalar_add(
                        kt_sb[:, hp, nsl], ps, bk_sb[:, hp:hp + 1])
                for sv in range(SB // 128):
                    st_i = nb * (SB // 128) + sv
                    ps = psum.tile([128, SB], dt.float32, tag="mm", bufs=2,
                                   name="ps")[:, 0:GD]
                    for t in range(NDT):
                        nc.tensor.matmul(
                            ps, xv[:, t, sv * 128:(sv + 1) * 128],
                            wv_sb[:, t, :],
                            start=(t == 0), stop=(t == NDT - 1))
                    nc.vector.tensor_copy(
                        va_sb[:, st_i, :, 0:DK],
                        ps.rearrange("p (h d) -> p h d", h=4))

            # ---- Q projections (all q-blocks) ---------------------------
            for qb in range(NSB):
                qsl = slice(qb * SB, (qb + 1) * SB)
                xq = xin.tile([128, NDT, SB], dt.bfloat16, tag="xq", name="xq")
                nc.sync.dma_start(xq, qTp[:, :, qsl])
                for hp in range(NPAIR):
                    ps = psum.tile([128, SB], dt.float32, tag="mm", bufs=2,
                                   name="ps")
                    for t in range(NDT):
                        nc.tensor.matmul(
                            ps, wq_sb[:, t, hp * 128:(hp + 1) * 128],
                            xq[:, t, :],
                            start=(t == 0), stop=(t == NDT - 1))
                    nc.vector.tensor_scalar_add(
                        qt_sb[:, hp, qsl], ps, bq_sb[:, hp:hp + 1])

            # ---- per q-block: attention -> out proj ---------------------
            for qb in range(NSB):
                qsl = slice(qb * SB, (qb + 1) * SB)
                # attention: scores^T [k, q] for both heads of the pair on
                # disjoint PE row-groups into one 2-bank PSUM tile; one wide
                # exp (PSUM -> SBUF bf16); PV accumulates yT_aug[65, q] with
                # lhsT = [V_h | 1] so row 64 = sum(exp).
                for hp in range(NPAIR):
                    ya = psum.tile([DK + 1, 2, SB], dt.float32, tag="ya",
                                   bufs=1, name="ya")
                    for kk in range(NKT):
                        ksl = slice(kk * 128, (kk + 1) * 128)
                        st = psum.tile([128, 2, SB], dt.float32, tag="st",
                                       bufs=2, name="st")
                        nc.tensor.matmul(
                            st[:, 0, :],
                            kt_sb[0:DK, hp, ksl], qt_sb[0:DK, hp, qsl],
                            start=True, stop=True, tile_position=(0, 0))
                        nc.tensor.matmul(
                            st[:, 1, :],
                            kt_sb[DK:128, hp, ksl], qt_sb[DK:128, hp, qsl],
                            start=True, stop=True, tile_position=(64, 0))
                        et = estage.tile([128, 2, SB], dt.bfloat16, tag="et",
                                         name="et")
                        nc.scalar.activation(et[:], st[:], Exp)
                        for j in range(2):
                            nc.tensor.matmul(
                                ya[:, j, :], va_sb[:, kk, 2 * hp + j, :],
                                et[:, j, :],
                                start=(kk == 0), stop=(kk == NKT - 1))
                    # normalize: y[0:64] * (1/y[64]) broadcast along free
                    # dim. 1/Z = exp(-log(Z)) on ScalarE (stays in the exp
                    # table set; DVE reciprocal is 8 cyc/elem on one lane).
                    lz = norm.tile([1, 2, SB], dt.float32, tag="lz")
                    nc.scalar.activation(lz[:], ya[DK:DK + 1, :, :], Log)
                    rr = norm.tile([1, 2, SB], dt.float32, tag="rr")
                    nc.scalar.activation(rr[:], lz[:], Exp, scale=-1.0)
                    for j in range(2):
                        rb = norm.tile([DK, SB], dt.float32, tag="rb")
                        nc.gpsimd.partition_broadcast(rb, rr[:, j, :])
                        nc.vector.tensor_mul(
                            yt_sb[j * DK:(j + 1) * DK, hp, qsl],
                            ya[0:DK, j, :], rb)

                # out[s, :] = sum_hp yT[:, hp, s-tile].T @ woT[hp]
                for sv in range(SB // 128):
                    st_i = qb * (SB // 128) + sv
                    ssl = slice(st_i * 128, (st_i + 1) * 128)
                    ot = ostage.tile([128, D], dt.float32, tag="ot", name="ot")
                    for nb2 in range(D // SB):
                        ps = psum.tile([128, SB], dt.float32, tag="mm", bufs=2,
                                       name="ps")
                        for hp in range(NPAIR):
                            nc.tensor.matmul(
                                ps, yt_sb[:, hp, ssl],
                                wo_sb[:, hp, nb2 * SB:(nb2 + 1) * SB],
                                start=(hp == 0), stop=(hp == NPAIR - 1))
                        nc.vector.tensor_copy(
                            ot[:, nb2 * SB:(nb2 + 1) * SB], ps)
                    nc.sync.dma_start(out[ssl, :], ot)

    nc.compile()
    return nc


def _get_program():
    if "nc" not in _CACHE:
        _CACHE["nc"] = _build_program()
    return _CACHE["nc"]


def make_in_maps(q, k, v, w_q, b_q, w_k, b_k, w_v, b_v, w_o, b_o):
    import ml_dtypes
    bf16 = ml_dtypes.bfloat16
    scale = 1.0 / np.sqrt(np.float32(DK))

    wqT = np.ascontiguousarray(w_q.T * scale)
    wkT = np.ascontiguousarray(w_k.T)
    wvT = np.ascontiguousarray(w_v.T)
    woT = np.ascontiguousarray(w_o.T)

    in_maps = []
    for b in range(B):
        qT = np.ascontiguousarray(q[b].T).astype(bf16)
        kT = np.ascontiguousarray(k[b].T).astype(bf16)
        vT = np.ascontiguousarray(v[b].T).astype(bf16)
        for g in range(NGROUPS):
            sl = slice(g * GD, (g + 1) * GD)
            in_maps.append({
                "qT": qT, "kT": kT, "vT": vT,
                "wqT": np.ascontiguousarray(wqT[:, sl]).astype(bf16),
                "wkT": np.ascontiguousarray(wkT[:, sl]).astype(bf16),
                "wvT": np.ascontiguousarray(wvT[:, sl]).astype(bf16),
                "woT": np.ascontiguousarray(woT[sl, :]).astype(bf16),
                "bq": np.ascontiguousarray(
                    (b_q[sl] * scale).reshape(NPAIR, 128).T).astype(np.float32),
                "bk": np.ascontiguousarray(
                    b_k[sl].reshape(NPAIR, 128).T).astype(np.float32),
            })
    return in_maps


def gather(results, w_o, b_v, b_o):
    corr = (b_v.astype(np.float64) @ w_o.T.astype(np.float64)
            + b_o.astype(np.float64)).astype(np.float32)
    out = np.empty((B, S, D), np.float32)
    for b in range(B):
        acc = np.zeros((S, D), np.float64)
        for g in range(NGROUPS):
            acc += results[b * NGROUPS + g]["out"]
        out[b] = acc.astype(np.float32) + corr
    return out


def kernel(q, k, v, w_q, b_q, w_k, b_k, w_v, b_v, w_o, b_o):
    from concourse.bass_utils import run_bass_kernel_spmd

    nc = _get_program()
    in_maps = make_in_maps(q, k, v, w_q, b_q, w_k, b_k, w_v, b_v, w_o, b_o)
    res = run_bass_kernel_spmd(nc, in_maps, list(range(NCORES)))
    return gather(res.results, w_o, b_v, b_o)


# revision 10
# speedup vs baseline: 1.1766x; 1.0676x over previous
"""Multi-head attention (B=2, S=2048, D=1024, H=16) on 8 Trainium2 cores.

Sharding: core = (batch b in {0,1}) x (head-group g in {0..3}, 4 heads each).
Each core computes its 4 heads end-to-end (Q/K/V projections restricted to the
group's 256 dims, attention, and the row-slice of the output projection) and
returns a partial [S, D] output; the host sums the 4 group partials per batch.

Device-side layouts (per core):
  qT/kT/vT  [D, S]   input activations, transposed on host, bf16
  wqT/wkT/wvT [D, 256] weight column-slices (wqT pre-scaled by 1/sqrt(dk)), bf16
  woT       [256, D] w_o.T row-slice, bf16
  bq/bk     [128, 2] per-partition bias ((b/8 for q), laid out [p, head-pair])
  out       [S, D]   fp32 partial output

b_v and b_o are handled exactly on the host: softmax rows sum to 1, so the
V-bias contributes b_v @ w_o.T + b_o as a constant row vector.
"""

import numpy as np

B, S, D = 2, 2048, 1024
H, DK = 16, 64
NCORES = 8
NGROUPS = 4                  # head-groups; 4 heads = 256 dims per group
GD = (H // NGROUPS) * DK     # 256 dims per group
NPAIR = 2                    # head-pairs per group (2 heads = 128 dims each)
SB = 512                     # s-block (matmul free dim / PSUM bank)
NSB = S // SB                # 4 s-blocks
NKT = S // 128               # 16 k-tiles of 128
NDT = D // 128               # 8 contraction tiles for projections
KG = 2                       # k-tiles per exp group (exp width = KG*512)

_CACHE = {}


def _build_program():
    from concourse import bacc, tile
    import concourse.mybir as mybir

    # Route both Exp and Ln to the combined natural_log_exp_and_others
    # activation-table set: the default first-match policy picks different
    # sets for Exp and Ln, reloading ACT tables (~2.7us) at every softmax
    # normalization. Stripping them from the other sets (order preserved, so
    # set ids stay valid) forces one load for the whole kernel.
    if not getattr(bacc, "_mha_act_tables_patched", False):
        _orig_tables = bacc.get_activation_tables

        def _patched_tables(arch):
            t = _orig_tables(arch)
            exp_ln = {mybir.ActivationFunctionType.Exp,
                      mybir.ActivationFunctionType.Ln}
            if any(exp_ln <= funcs for funcs in t.values()):
                for name, funcs in t.items():
                    if not exp_ln <= funcs:
                        t[name] = funcs - exp_ln
            return t

        bacc.get_activation_tables = _patched_tables
        bacc._mha_act_tables_patched = True

    dt = mybir.dt
    nc = bacc.Bacc("TRN2", target_bir_lowering=False, debug=False,
                   num_devices=NCORES)

    qT = nc.dram_tensor("qT", [D, S], dt.bfloat16, kind="ExternalInput").ap()
    kT = nc.dram_tensor("kT", [D, S], dt.bfloat16, kind="ExternalInput").ap()
    vT = nc.dram_tensor("vT", [D, S], dt.bfloat16, kind="ExternalInput").ap()
    wqT = nc.dram_tensor("wqT", [D, GD], dt.bfloat16, kind="ExternalInput").ap()
    wkT = nc.dram_tensor("wkT", [D, GD], dt.bfloat16, kind="ExternalInput").ap()
    wvT = nc.dram_tensor("wvT", [D, GD], dt.bfloat16, kind="ExternalInput").ap()
    woT = nc.dram_tensor("woT", [GD, D], dt.bfloat16, kind="ExternalInput").ap()
    bq = nc.dram_tensor("bq", [128, NPAIR], dt.float32, kind="ExternalInput").ap()
    bk = nc.dram_tensor("bk", [128, NPAIR], dt.float32, kind="ExternalInput").ap()
    out = nc.dram_tensor("out", [S, D], dt.float32, kind="ExternalOutput").ap()

    qT_t = qT.rearrange("(t p) s -> t p s", p=128)   # [8, 128, S]
    kT_t = kT.rearrange("(t p) s -> t p s", p=128)
    vT_t = vT.rearrange("(t p) s -> t p s", p=128)
    wqT_t = wqT.rearrange("(t p) m -> t p m", p=128)  # [8, 128, GD]
    wkT_t = wkT.rearrange("(t p) m -> t p m", p=128)
    wvT_t = wvT.rearrange("(t p) m -> t p m", p=128)
    woT_t = woT.rearrange("(t p) m -> t p m", p=128)  # [2, 128, D]

    Exp = mybir.ActivationFunctionType.Exp
    Log = mybir.ActivationFunctionType.Ln

    with tile.TileContext(nc) as tc:
        with (
            tc.tile_pool(name="const", bufs=1) as const,
            tc.tile_pool(name="xin", bufs=2) as xin,
            tc.tile_pool(name="acts", bufs=1) as acts,
            tc.tile_pool(name="estage", bufs=8) as estage,
            tc.tile_pool(name="norm", bufs=4) as norm,
            tc.tile_pool(name="ostage", bufs=3) as ostage,
            tc.tile_pool(name="psum", bufs=1, space="PSUM") as psum,
        ):
            # ---- resident constants -------------------------------------
            wq_sb = const.tile([128, NDT, GD], dt.bfloat16, tag="wq")
            wk_sb = const.tile([128, NDT, GD], dt.bfloat16, tag="wk")
            wv_sb = const.tile([128, NDT, GD], dt.bfloat16, tag="wv")
            wo_sb = const.tile([128, NPAIR, D], dt.bfloat16, tag="wo")
            bq_sb = const.tile([128, NPAIR], dt.float32, tag="bq")
            bk_sb = const.tile([128, NPAIR], dt.float32, tag="bk")
            nc.sync.dma_start(wq_sb[:], wqT.rearrange("(t p) m -> p t m", p=128))
            nc.sync.dma_start(wk_sb[:], wkT.rearrange("(t p) m -> p t m", p=128))
            nc.sync.dma_start(wv_sb[:], wvT.rearrange("(t p) m -> p t m", p=128))
            nc.sync.dma_start(wo_sb[:], woT.rearrange("(t p) m -> p t m", p=128))
            nc.sync.dma_start(bq_sb[:], bq)
            nc.sync.dma_start(bk_sb[:], bk)

            # ---- activation tiles (whole-group residents) ---------------
            # QT/KT: [d-within-pair(128), pair, s]; V_aug: [k(128), k-tile,
            # head(4), dk+1] with col 64 = ones (softmax denominator trick).
            qt_sb = acts.tile([128, NPAIR, S], dt.bfloat16, tag="qt")
            kt_sb = acts.tile([128, NPAIR, S], dt.bfloat16, tag="kt")
            va_sb = acts.tile([128, NKT, 4, DK + 1], dt.bfloat16, tag="va")
            yt_sb = acts.tile([128, NPAIR, S], dt.bfloat16, tag="yt")

            nc.vector.memset(va_sb[:, :, :, DK:DK + 1], 1.0)

            # ---- K / V projections, interleaved per s-block -------------
            # K:  KT[d_g, s] = sum_t wkT[t].T @ kT[t]   (+ bk per-partition)
            # V:  V[s, d_g]  = sum_t vT[t][:, s-tile].T @ wvT[t]
            kTp = kT.rearrange("(t p) s -> p t s", p=128)   # [128, 8, S]
            vTp = vT.rearrange("(t p) s -> p t s", p=128)
            qTp = qT.rearrange("(t p) s -> p t s", p=128)
            for nb in range(NSB):
                nsl = slice(nb * SB, (nb + 1) * SB)
                xk = xin.tile([128, NDT, SB], dt.bfloat16, tag="xk", name="xk")
                nc.sync.dma_start(xk, kTp[:, :, nsl])
                xv = xin.tile([128, NDT, SB], dt.bfloat16, tag="xv", name="xv")
                nc.sync.dma_start(xv, vTp[:, :, nsl])
                for hp in range(NPAIR):
                    ps = psum.tile([128, SB], dt.float32, tag="mm", bufs=2,
                                   name="ps")
                    for t in range(NDT):
                        nc.tensor.matmul(
                            ps, wk_sb[:, t, hp * 128:(hp + 1) * 128],
                            xk[:, t, :],
                            start=(t == 0), stop=(t == NDT - 1))
                    nc.vector.tensor_scalar_add(
                        kt_sb[:, hp, nsl], ps, bk_sb[:, hp:hp + 1])
                for sv in range(SB // 128):
                    st_i = nb * (SB // 128) + sv
                    ps = psum.tile([128, SB], dt.float32, tag="mm", bufs=2,
                                   name="ps")[:, 0:GD]
                    for t in range(NDT):
                        nc.tensor.matmul(
                            ps, xv[:, t, sv * 128:(sv + 1) * 128],
                            wv_sb[:, t, :],
                            start=(t == 0), stop=(t == NDT - 1))
                    nc.vector.tensor_copy(
                        va_sb[:, st_i, :, 0:DK],
                        ps.rearrange("p (h d) -> p h d", h=4))

            # ---- Q projections (all q-blocks) ---------------------------
            for qb in range(NSB):
                qsl = slice(qb * SB, (qb + 1) * SB)
                xq = xin.tile([128, NDT, SB], dt.bfloat16, tag="xq", name="xq")
                nc.sync.dma_start(xq, qTp[:, :, qsl])
                for hp in range(NPAIR):
                    ps = psum.tile([128, SB], dt.float32, tag="mm", bufs=2,
                                   name="ps")
                    for t in range(NDT):
                        nc.tensor.matmul(
                            ps, wq_sb[:, t, hp * 128:(hp + 1) * 128],
                            xq[:, t, :],
                            start=(t == 0), stop=(t == NDT - 1))
                    nc.vector.tensor_scalar_add(
                        qt_sb[:, hp, qsl], ps, bq_sb[:, hp:hp + 1])

            # ---- per q-block: attention -> out proj ---------------------
            for qb in range(NSB):
                qsl = slice(qb * SB, (qb + 1) * SB)
                # attention: scores^T [k, q] for both heads of the pair on
                # disjoint PE row-groups into one 2-bank PSUM tile; one wide
                # exp (PSUM -> SBUF bf16); PV accumulates yT_aug[65, q] with
                # lhsT = [V_h | 1] so row 64 = sum(exp).
                for hp in range(NPAIR):
                    ya = psum.tile([DK + 1, 2, SB], dt.float32, tag="ya",
                                   bufs=1, name="ya")
                    for kk in range(NKT):
                        ksl = slice(kk * 128, (kk + 1) * 128)
                        st = psum.tile([128, 2, SB], dt.float32, tag="st",
                                       bufs=2, name="st")
                        nc.tensor.matmul(
                            st[:, 0, :],
                            kt_sb[0:DK, hp, ksl], qt_sb[0:DK, hp, qsl],
                            start=True, stop=True, tile_position=(0, 0))
                        nc.tensor.matmul(
                            st[:, 1, :],
                            kt_sb[DK:128, hp, ksl], qt_sb[DK:128, hp, qsl],
                            start=True, stop=True, tile_position=(64, 0))
                        et = estage.tile([128, 2, SB], dt.bfloat16, tag="et",
                                         name="et")
                        nc.scalar.activation(et[:], st[:], Exp)
                        for j in range(2):
                            nc.tensor.matmul(
                                ya[:, j, :], va_sb[:, kk, 2 * hp + j, :],
                                et[:, j, :],
                                start=(kk == 0), stop=(kk == NKT - 1))
                    # normalize: y[0:64] * (1/y[64]) broadcast along free
                    # dim. 1/Z = exp(-log(Z)) on ScalarE (stays in the exp
                    # table set; DVE reciprocal is 8 cyc/elem on one lane).
                    lz = norm.tile([1, 2, SB], dt.float32, tag="lz")
                    nc.scalar.activation(lz[:], ya[DK:DK + 1, :, :], Log)
                    rr = norm.tile([1, 2, SB], dt.float32, tag="rr")
                    nc.scalar.activation(rr[:], lz[:], Exp, scale=-1.0)
                    for j in range(2):
                        rb = norm.tile([DK, SB], dt.float32, tag="rb")
                        nc.gpsimd.partition_broadcast(rb, rr[:, j, :])
                        nc.vector.tensor_mul(
                            yt_sb[j * DK:(j + 1) * DK, hp, qsl],
                            ya[0:DK, j, :], rb)

                # out[s, :] = sum_hp yT[:, hp, s-tile].T @ woT[hp]
                for sv in range(SB // 128):
                    st_i = qb * (SB // 128) + sv
                    ssl = slice(st_i * 128, (st_i + 1) * 128)
                    ot = ostage.tile([128, D], dt.float32, tag="ot", name="ot")
                    for nb2 in range(D // SB):
                        ps = psum.tile([128, SB], dt.float32, tag="mm", bufs=2,
                                       name="ps")
                        for hp in range(NPAIR):
                            nc.tensor.matmul(
                                ps, yt_sb[:, hp, ssl],
                                wo_sb[:, hp, nb2 * SB:(nb2 + 1) * SB],
                                start=(hp == 0), stop=(hp == NPAIR - 1))
                        nc.vector.tensor_copy(
                            ot[:, nb2 * SB:(nb2 + 1) * SB], ps)
                    nc.sync.dma_start(out[ssl, :], ot)

    nc.compile()
    return nc


def _get_program():
    if "nc" not in _CACHE:
        _CACHE["nc"] = _build_program()
    return _CACHE["nc"]


def make_in_maps(q, k, v, w_q, b_q, w_k, b_k, w_v, b_v, w_o, b_o):
    import ml_dtypes
    bf16 = ml_dtypes.bfloat16
    scale = 1.0 / np.sqrt(np.float32(DK))

    wqT = np.ascontiguousarray(w_q.T * scale)
    wkT = np.ascontiguousarray(w_k.T)
    wvT = np.ascontiguousarray(w_v.T)
    woT = np.ascontiguousarray(w_o.T)

    in_maps = []
    for b in range(B):
        qT = np.ascontiguousarray(q[b].T).astype(bf16)
        kT = np.ascontiguousarray(k[b].T).astype(bf16)
        vT = np.ascontiguousarray(v[b].T).astype(bf16)
        for g in range(NGROUPS):
            sl = slice(g * GD, (g + 1) * GD)
            in_maps.append({
                "qT": qT, "kT": kT, "vT": vT,
                "wqT": np.ascontiguousarray(wqT[:, sl]).astype(bf16),
                "wkT": np.ascontiguousarray(wkT[:, sl]).astype(bf16),
                "wvT": np.ascontiguousarray(wvT[:, sl]).astype(bf16),
                "woT": np.ascontiguousarray(woT[sl, :]).astype(bf16),
                "bq": np.ascontiguousarray(
                    (b_q[sl] * scale).reshape(NPAIR, 128).T).astype(np.float32),
                "bk": np.ascontiguousarray(
                    b_k[sl].reshape(NPAIR, 128).T).astype(np.float32),
            })
    return in_maps


def gather(results, w_o, b_v, b_o):
    corr = (b_v.astype(np.float64) @ w_o.T.astype(np.float64)
            + b_o.astype(np.float64)).astype(np.float32)
    out = np.empty((B, S, D), np.float32)
    for b in range(B):
        acc = np.zeros((S, D), np.float64)
        for g in range(NGROUPS):
            acc += results[b * NGROUPS + g]["out"]
        out[b] = acc.astype(np.float32) + corr
    return out


def kernel(q, k, v, w_q, b_q, w_k, b_k, w_v, b_v, w_o, b_o):
    from concourse.bass_utils import run_bass_kernel_spmd

    nc = _get_program()
    in_maps = make_in_maps(q, k, v, w_q, b_q, w_k, b_k, w_v, b_v, w_o, b_o)
    res = run_bass_kernel_spmd(nc, in_maps, list(range(NCORES)))
    return gather(res.results, w_o, b_v, b_o)


# revision 11
# speedup vs baseline: 1.2878x; 1.0945x over previous
"""Multi-head attention (B=2, S=2048, D=1024, H=16) on 8 Trainium2 cores.

Sharding: core = (batch b in {0,1}) x (head-group g in {0..3}, 4 heads each).
Each core computes its 4 heads end-to-end (Q/K/V projections restricted to the
group's 256 dims, attention, and the row-slice of the output projection) and
returns a partial [S, D] output; the host sums the 4 group partials per batch.

Device-side layouts (per core):
  qT/kT/vT  [D, S]   input activations, transposed on host, bf16
  wqT/wkT/wvT [D, 256] weight column-slices (wqT pre-scaled by 1/sqrt(dk)), bf16
  woT       [256, D] w_o.T row-slice, bf16
  bq/bk     [128, 2] per-partition bias ((b/8 for q), laid out [p, head-pair])
  out       [S, D]   fp32 partial output

b_v and b_o are handled exactly on the host: softmax rows sum to 1, so the
V-bias contributes b_v @ w_o.T + b_o as a constant row vector.
"""

import numpy as np

B, S, D = 2, 2048, 1024
H, DK = 16, 64
NCORES = 8
NGROUPS = 4                  # head-groups; 4 heads = 256 dims per group
GD = (H // NGROUPS) * DK     # 256 dims per group
NPAIR = 2                    # head-pairs per group (2 heads = 128 dims each)
SB = 512                     # s-block (matmul free dim / PSUM bank)
NSB = S // SB                # 4 s-blocks
NKT = S // 128               # 16 k-tiles of 128
NDT = D // 128               # 8 contraction tiles for projections
KG = 2                       # k-tiles per exp group (exp width = KG*512)

_CACHE = {}


def _build_program():
    from concourse import bacc, tile
    import concourse.mybir as mybir

    # Route both Exp and Ln to the combined natural_log_exp_and_others
    # activation-table set: the default first-match policy picks different
    # sets for Exp and Ln, reloading ACT tables (~2.7us) at every softmax
    # normalization. Stripping them from the other sets (order preserved, so
    # set ids stay valid) forces one load for the whole kernel.
    if not getattr(bacc, "_mha_act_tables_patched", False):
        _orig_tables = bacc.get_activation_tables

        def _patched_tables(arch):
            t = _orig_tables(arch)
            exp_ln = {mybir.ActivationFunctionType.Exp,
                      mybir.ActivationFunctionType.Ln}
            if any(exp_ln <= funcs for funcs in t.values()):
                for name, funcs in t.items():
                    if not exp_ln <= funcs:
                        t[name] = funcs - exp_ln
            return t

        bacc.get_activation_tables = _patched_tables
        bacc._mha_act_tables_patched = True

    dt = mybir.dt
    nc = bacc.Bacc("TRN2", target_bir_lowering=False, debug=False,
                   num_devices=NCORES)

    qT = nc.dram_tensor("qT", [D, S], dt.bfloat16, kind="ExternalInput").ap()
    kT = nc.dram_tensor("kT", [D, S], dt.bfloat16, kind="ExternalInput").ap()
    vT = nc.dram_tensor("vT", [D, S], dt.bfloat16, kind="ExternalInput").ap()
    wqT = nc.dram_tensor("wqT", [D, GD], dt.bfloat16, kind="ExternalInput").ap()
    wkT = nc.dram_tensor("wkT", [D, GD], dt.bfloat16, kind="ExternalInput").ap()
    wvT = nc.dram_tensor("wvT", [D, GD], dt.bfloat16, kind="ExternalInput").ap()
    woT = nc.dram_tensor("woT", [GD, D], dt.bfloat16, kind="ExternalInput").ap()
    bq = nc.dram_tensor("bq", [128, NPAIR], dt.float32, kind="ExternalInput").ap()
    bk = nc.dram_tensor("bk", [128, NPAIR], dt.float32, kind="ExternalInput").ap()
    out = nc.dram_tensor("out", [S, D], dt.float32, kind="ExternalOutput").ap()

    qT_t = qT.rearrange("(t p) s -> t p s", p=128)   # [8, 128, S]
    kT_t = kT.rearrange("(t p) s -> t p s", p=128)
    vT_t = vT.rearrange("(t p) s -> t p s", p=128)
    wqT_t = wqT.rearrange("(t p) m -> t p m", p=128)  # [8, 128, GD]
    wkT_t = wkT.rearrange("(t p) m -> t p m", p=128)
    wvT_t = wvT.rearrange("(t p) m -> t p m", p=128)
    woT_t = woT.rearrange("(t p) m -> t p m", p=128)  # [2, 128, D]

    Exp = mybir.ActivationFunctionType.Exp
    Log = mybir.ActivationFunctionType.Ln

    with tile.TileContext(nc) as tc:
        with (
            tc.tile_pool(name="const", bufs=1) as const,
            tc.tile_pool(name="xin", bufs=2) as xin,
            tc.tile_pool(name="acts", bufs=1) as acts,
            tc.tile_pool(name="estage", bufs=8) as estage,
            tc.tile_pool(name="norm", bufs=4) as norm,
            tc.tile_pool(name="ostage", bufs=3) as ostage,
            tc.tile_pool(name="psum", bufs=1, space="PSUM") as psum,
        ):
            # ---- resident constants -------------------------------------
            wq_sb = const.tile([128, NDT, GD], dt.bfloat16, tag="wq")
            wk_sb = const.tile([128, NDT, GD], dt.bfloat16, tag="wk")
            wv_sb = const.tile([128, NDT, GD], dt.bfloat16, tag="wv")
            wo_sb = const.tile([128, NPAIR, D], dt.bfloat16, tag="wo")
            bq_sb = const.tile([128, NPAIR], dt.float32, tag="bq")
            bk_sb = const.tile([128, NPAIR], dt.float32, tag="bk")
            nc.sync.dma_start(wq_sb[:], wqT.rearrange("(t p) m -> p t m", p=128))
            nc.sync.dma_start(wk_sb[:], wkT.rearrange("(t p) m -> p t m", p=128))
            nc.sync.dma_start(wv_sb[:], wvT.rearrange("(t p) m -> p t m", p=128))
            nc.sync.dma_start(wo_sb[:], woT.rearrange("(t p) m -> p t m", p=128))
            nc.sync.dma_start(bq_sb[:], bq)
            nc.sync.dma_start(bk_sb[:], bk)

            # ---- activation tiles (whole-group residents) ---------------
            # QT/KT: [d-within-pair(128), pair, s]; V_aug: [k(128), k-tile,
            # head(4), dk+1] with col 64 = ones (softmax denominator trick).
            qt_sb = acts.tile([128, NPAIR, S], dt.bfloat16, tag="qt")
            kt_sb = acts.tile([128, NPAIR, S], dt.bfloat16, tag="kt")
            va_sb = acts.tile([128, NKT, 4, DK + 1], dt.bfloat16, tag="va")
            yt_sb = acts.tile([128, NPAIR, S], dt.bfloat16, tag="yt")

            nc.vector.memset(va_sb[:, :, :, DK:DK + 1], 1.0)

            # ---- K / V projections, interleaved per s-block -------------
            # K:  KT[d_g, s] = sum_t wkT[t].T @ kT[t]   (+ bk per-partition)
            # V:  V[s, d_g]  = sum_t vT[t][:, s-tile].T @ wvT[t]
            kTp = kT.rearrange("(t p) s -> p t s", p=128)   # [128, 8, S]
            vTp = vT.rearrange("(t p) s -> p t s", p=128)
            qTp = qT.rearrange("(t p) s -> p t s", p=128)
            for nb in range(NSB):
                nsl = slice(nb * SB, (nb + 1) * SB)
                xk = xin.tile([128, NDT, SB], dt.bfloat16, tag="xk", name="xk")
                nc.sync.dma_start(xk, kTp[:, :, nsl])
                xv = xin.tile([128, NDT, SB], dt.bfloat16, tag="xv", name="xv")
                nc.sync.dma_start(xv, vTp[:, :, nsl])
                xq = xin.tile([128, NDT, SB], dt.bfloat16, tag="xq", name="xq")
                nc.sync.dma_start(xq, qTp[:, :, nsl])
                for hp in range(NPAIR):
                    ps = psum.tile([128, SB], dt.float32, tag="mm", bufs=2,
                                   name="ps")
                    for t in range(NDT):
                        nc.tensor.matmul(
                            ps, wk_sb[:, t, hp * 128:(hp + 1) * 128],
                            xk[:, t, :],
                            start=(t == 0), stop=(t == NDT - 1))
                    nc.vector.tensor_scalar_add(
                        kt_sb[:, hp, nsl], ps, bk_sb[:, hp:hp + 1])
                for sv in range(SB // 128):
                    st_i = nb * (SB // 128) + sv
                    ps = psum.tile([128, SB], dt.float32, tag="mm", bufs=2,
                                   name="ps")[:, 0:GD]
                    for t in range(NDT):
                        nc.tensor.matmul(
                            ps, xv[:, t, sv * 128:(sv + 1) * 128],
                            wv_sb[:, t, :],
                            start=(t == 0), stop=(t == NDT - 1))
                    nc.vector.tensor_copy(
                        va_sb[:, st_i, :, 0:DK],
                        ps.rearrange("p (h d) -> p h d", h=4))
                for hp in range(NPAIR):
                    ps = psum.tile([128, SB], dt.float32, tag="mm", bufs=2,
                                   name="ps")
                    for t in range(NDT):
                        nc.tensor.matmul(
                            ps, wq_sb[:, t, hp * 128:(hp + 1) * 128],
                            xq[:, t, :],
                            start=(t == 0), stop=(t == NDT - 1))
                    nc.vector.tensor_scalar_add(
                        qt_sb[:, hp, nsl], ps, bq_sb[:, hp:hp + 1])

            # ---- per q-block: attention -> out proj ---------------------
            for qb in range(NSB):
                qsl = slice(qb * SB, (qb + 1) * SB)
                # attention: scores^T [k, q] for both heads of the pair on
                # disjoint PE row-groups into one 2-bank PSUM tile; one wide
                # exp (PSUM -> SBUF bf16); PV accumulates yT_aug[65, q] with
                # lhsT = [V_h | 1] so row 64 = sum(exp).
                for hp in range(NPAIR):
                    ya = psum.tile([DK + 1, 2, SB], dt.float32, tag="ya",
                                   bufs=1, name="ya")
                    for kk in range(NKT):
                        ksl = slice(kk * 128, (kk + 1) * 128)
                        st = psum.tile([128, 2, SB], dt.float32, tag="st",
                                       bufs=2, name="st")
                        nc.tensor.matmul(
                            st[:, 0, :],
                            kt_sb[0:DK, hp, ksl], qt_sb[0:DK, hp, qsl],
                            start=True, stop=True, tile_position=(0, 0))
                        nc.tensor.matmul(
                            st[:, 1, :],
                            kt_sb[DK:128, hp, ksl], qt_sb[DK:128, hp, qsl],
                            start=True, stop=True, tile_position=(64, 0))
                        et = estage.tile([128, 2, SB], dt.bfloat16, tag="et",
                                         name="et")
                        nc.scalar.activation(et[:], st[:], Exp)
                        for j in range(2):
                            nc.tensor.matmul(
                                ya[:, j, :], va_sb[:, kk, 2 * hp + j, :],
                                et[:, j, :],
                                start=(kk == 0), stop=(kk == NKT - 1))
                    # Copy yT_aug out of PSUM immediately (frees the ya
                    # bank for the next pair-block), then normalize off the
                    # critical path: 1/Z = exp(-ln(Z)) on ScalarE (stays in
                    # the exp table set; DVE reciprocal is 8 cyc/elem on a
                    # single lane), partition-broadcast on GpSimd, multiply
                    # on VectorE.
                    ys = norm.tile([DK + 1, 2, SB], dt.float32, tag="ys",
                                   bufs=3)
                    nc.vector.tensor_copy(ys[:], ya[:])
                    lz = norm.tile([1, 2, SB], dt.float32, tag="lz")
                    nc.scalar.activation(lz[:], ys[DK:DK + 1, :, :], Log)
                    rr = norm.tile([1, 2, SB], dt.float32, tag="rr")
                    nc.scalar.activation(rr[:], lz[:], Exp, scale=-1.0)
                    for j in range(2):
                        rb = norm.tile([DK, SB], dt.float32, tag="rb")
                        nc.gpsimd.partition_broadcast(rb, rr[:, j, :])
                        nc.vector.tensor_mul(
                            yt_sb[j * DK:(j + 1) * DK, hp, qsl],
                            ys[0:DK, j, :], rb)

            # ---- output projection (fills PE slack of the ACT-bound
            # attention stream; emitted last so scores keep priority) ------
            if True:
                for st_i in range(S // 128):
                    ssl = slice(st_i * 128, (st_i + 1) * 128)
                    ot = ostage.tile([128, D], dt.float32, tag="ot", name="ot")
                    for nb2 in range(D // SB):
                        ps = psum.tile([128, SB], dt.float32, tag="mm", bufs=2,
                                       name="ps")
                        for hp in range(NPAIR):
                            nc.tensor.matmul(
                                ps, yt_sb[:, hp, ssl],
                                wo_sb[:, hp, nb2 * SB:(nb2 + 1) * SB],
                                start=(hp == 0), stop=(hp == NPAIR - 1))
                        nc.vector.tensor_copy(
                            ot[:, nb2 * SB:(nb2 + 1) * SB], ps)
                    nc.sync.dma_start(out[ssl, :], ot)

    nc.compile()
    return nc


def _get_program():
    if "nc" not in _CACHE:
        _CACHE["nc"] = _build_program()
    return _CACHE["nc"]


def make_in_maps(q, k, v, w_q, b_q, w_k, b_k, w_v, b_v, w_o, b_o):
    import ml_dtypes
    bf16 = ml_dtypes.bfloat16
    scale = 1.0 / np.sqrt(np.float32(DK))

    wqT = np.ascontiguousarray(w_q.T * scale)
    wkT = np.ascontiguousarray(w_k.T)
    wvT = np.ascontiguousarray(w_v.T)
    woT = np.ascontiguousarray(w_o.T)

    in_maps = []
    for b in range(B):
        qT = np.ascontiguousarray(q[b].T).astype(bf16)
        kT = np.ascontiguousarray(k[b].T).astype(bf16)
        vT = np.ascontiguousarray(v[b].T).astype(bf16)
        for g in range(NGROUPS):
            sl = slice(g * GD, (g + 1) * GD)
            in_maps.append({
                "qT": qT, "kT": kT, "vT": vT,
                "wqT": np.ascontiguousarray(wqT[:, sl]).astype(bf16),
                "wkT": np.ascontiguousarray(wkT[:, sl]).astype(bf16),
                "wvT": np.ascontiguousarray(wvT[:, sl]).astype(bf16),
                "woT": np.ascontiguousarray(woT[sl, :]).astype(bf16),
                "bq": np.ascontiguousarray(
                    (b_q[sl] * scale).reshape(NPAIR, 128).T).astype(np.float32),
                "bk": np.ascontiguousarray(
                    b_k[sl].reshape(NPAIR, 128).T).astype(np.float32),
            })
    return in_maps


def gather(results, w_o, b_v, b_o):
    corr = (b_v.astype(np.float64) @ w_o.T.astype(np.float64)
            + b_o.astype(np.float64)).astype(np.float32)
    out = np.empty((B, S, D), np.float32)
    for b in range(B):
        acc = np.zeros((S, D), np.float64)
        for g in range(NGROUPS):
            acc += results[b * NGROUPS + g]["out"]
        out[b] = acc.astype(np.float32) + corr
    return out


def kernel(q, k, v, w_q, b_q, w_k, b_k, w_v, b_v, w_o, b_o):
    from concourse.bass_utils import run_bass_kernel_spmd

    nc = _get_program()
    in_maps = make_in_maps(q, k, v, w_q, b_q, w_k, b_k, w_v, b_v, w_o, b_o)
    res = run_bass_kernel_spmd(nc, in_maps, list(range(NCORES)))
    return gather(res.results, w_o, b_v, b_o)


# revision 12
# speedup vs baseline: 1.3130x; 1.0195x over previous
"""Multi-head attention (B=2, S=2048, D=1024, H=16) on 8 Trainium2 cores.

Sharding: core = (batch b in {0,1}) x (head-group g in {0..3}, 4 heads each).
Each core computes its 4 heads end-to-end (Q/K/V projections restricted to the
group's 256 dims, attention, and the row-slice of the output projection) and
returns a partial [S, D] output; the host sums the 4 group partials per batch.

Device-side layouts (per core):
  qT/kT/vT  [D, S]   input activations, transposed on host, bf16
  wqT/wkT/wvT [D, 256] weight column-slices (wqT pre-scaled by 1/sqrt(dk)), bf16
  woT       [256, D] w_o.T row-slice, bf16
  bq/bk     [128, 2] per-partition bias ((b/8 for q), laid out [p, head-pair])
  out       [S, D]   fp32 partial output

b_v and b_o are handled exactly on the host: softmax rows sum to 1, so the
V-bias contributes b_v @ w_o.T + b_o as a constant row vector.
"""

import numpy as np

B, S, D = 2, 2048, 1024
H, DK = 16, 64
NCORES = 8
NGROUPS = 4                  # head-groups; 4 heads = 256 dims per group
GD = (H // NGROUPS) * DK     # 256 dims per group
NPAIR = 2                    # head-pairs per group (2 heads = 128 dims each)
SB = 512                     # s-block (matmul free dim / PSUM bank)
NSB = S // SB                # 4 s-blocks
NKT = S // 128               # 16 k-tiles of 128
NDT = D // 128               # 8 contraction tiles for projections
KG = 2                       # k-tiles per exp group (exp width = KG*512)

_CACHE = {}


def _build_program():
    from concourse import bacc, tile
    import concourse.mybir as mybir

    # Route both Exp and Ln to the combined natural_log_exp_and_others
    # activation-table set: the default first-match policy picks different
    # sets for Exp and Ln, reloading ACT tables (~2.7us) at every softmax
    # normalization. Stripping them from the other sets (order preserved, so
    # set ids stay valid) forces one load for the whole kernel.
    if not getattr(bacc, "_mha_act_tables_patched", False):
        _orig_tables = bacc.get_activation_tables

        def _patched_tables(arch):
            t = _orig_tables(arch)
            exp_ln = {mybir.ActivationFunctionType.Exp,
                      mybir.ActivationFunctionType.Ln}
            if any(exp_ln <= funcs for funcs in t.values()):
                for name, funcs in t.items():
                    if not exp_ln <= funcs:
                        t[name] = funcs - exp_ln
            return t

        bacc.get_activation_tables = _patched_tables
        bacc._mha_act_tables_patched = True

    dt = mybir.dt
    nc = bacc.Bacc("TRN2", target_bir_lowering=False, debug=False,
                   num_devices=NCORES)

    qT = nc.dram_tensor("qT", [D, S], dt.bfloat16, kind="ExternalInput").ap()
    kT = nc.dram_tensor("kT", [D, S], dt.bfloat16, kind="ExternalInput").ap()
    vT = nc.dram_tensor("vT", [D, S], dt.bfloat16, kind="ExternalInput").ap()
    wqT = nc.dram_tensor("wqT", [D, GD], dt.bfloat16, kind="ExternalInput").ap()
    wkT = nc.dram_tensor("wkT", [D, GD], dt.bfloat16, kind="ExternalInput").ap()
    wvT = nc.dram_tensor("wvT", [D, GD], dt.bfloat16, kind="ExternalInput").ap()
    woT = nc.dram_tensor("woT", [GD, D], dt.bfloat16, kind="ExternalInput").ap()
    bq = nc.dram_tensor("bq", [128, NPAIR], dt.float32, kind="ExternalInput").ap()
    bk = nc.dram_tensor("bk", [128, NPAIR], dt.float32, kind="ExternalInput").ap()
    out = nc.dram_tensor("out", [S, D], dt.float32, kind="ExternalOutput").ap()

    qT_t = qT.rearrange("(t p) s -> t p s", p=128)   # [8, 128, S]
    kT_t = kT.rearrange("(t p) s -> t p s", p=128)
    vT_t = vT.rearrange("(t p) s -> t p s", p=128)
    wqT_t = wqT.rearrange("(t p) m -> t p m", p=128)  # [8, 128, GD]
    wkT_t = wkT.rearrange("(t p) m -> t p m", p=128)
    wvT_t = wvT.rearrange("(t p) m -> t p m", p=128)
    woT_t = woT.rearrange("(t p) m -> t p m", p=128)  # [2, 128, D]

    Exp = mybir.ActivationFunctionType.Exp
    Log = mybir.ActivationFunctionType.Ln

    with tile.TileContext(nc) as tc:
        with (
            tc.tile_pool(name="const", bufs=1) as const,
            tc.tile_pool(name="xin", bufs=2) as xin,
            tc.tile_pool(name="acts", bufs=1) as acts,
            tc.tile_pool(name="estage", bufs=8) as estage,
            tc.tile_pool(name="norm", bufs=4) as norm,
            tc.tile_pool(name="ostage", bufs=3) as ostage,
            tc.tile_pool(name="psum", bufs=1, space="PSUM") as psum,
        ):
            # ---- resident constants -------------------------------------
            wq_sb = const.tile([128, NDT, GD], dt.bfloat16, tag="wq")
            wk_sb = const.tile([128, NDT, GD], dt.bfloat16, tag="wk")
            wv_sb = const.tile([128, NDT, GD], dt.bfloat16, tag="wv")
            wo_sb = const.tile([128, NPAIR, D], dt.bfloat16, tag="wo")
            bq_sb = const.tile([128, NPAIR], dt.float32, tag="bq")
            bk_sb = const.tile([128, NPAIR], dt.float32, tag="bk")
            nc.sync.dma_start(wk_sb[:], wkT.rearrange("(t p) m -> p t m", p=128))
            nc.sync.dma_start(bk_sb[:], bk)
            nc.sync.dma_start(wv_sb[:], wvT.rearrange("(t p) m -> p t m", p=128))
            nc.sync.dma_start(wq_sb[:], wqT.rearrange("(t p) m -> p t m", p=128))
            nc.sync.dma_start(bq_sb[:], bq)
            nc.sync.dma_start(wo_sb[:], woT.rearrange("(t p) m -> p t m", p=128))

            # ---- activation tiles (whole-group residents) ---------------
            # QT/KT: [d-within-pair(128), pair, s]; V_aug: [k(128), k-tile,
            # head(4), dk+1] with col 64 = ones (softmax denominator trick).
            qt_sb = acts.tile([128, NPAIR, S], dt.bfloat16, tag="qt")
            kt_sb = acts.tile([128, NPAIR, S], dt.bfloat16, tag="kt")
            va_sb = acts.tile([128, NKT, 4, DK + 1], dt.bfloat16, tag="va")
            yt_sb = acts.tile([128, NPAIR, S], dt.bfloat16, tag="yt")

            nc.vector.memset(va_sb[:, :, :, DK:DK + 1], 1.0)

            # ---- K / V projections, interleaved per s-block -------------
            # K:  KT[d_g, s] = sum_t wkT[t].T @ kT[t]   (+ bk per-partition)
            # V:  V[s, d_g]  = sum_t vT[t][:, s-tile].T @ wvT[t]
            kTp = kT.rearrange("(t p) s -> p t s", p=128)   # [128, 8, S]
            vTp = vT.rearrange("(t p) s -> p t s", p=128)
            qTp = qT.rearrange("(t p) s -> p t s", p=128)
            for nb in range(NSB):
                nsl = slice(nb * SB, (nb + 1) * SB)
                xk = xin.tile([128, NDT, SB], dt.bfloat16, tag="xk", name="xk")
                nc.sync.dma_start(xk, kTp[:, :, nsl])
                xv = xin.tile([128, NDT, SB], dt.bfloat16, tag="xv", name="xv")
                nc.sync.dma_start(xv, vTp[:, :, nsl])
                xq = xin.tile([128, NDT, SB], dt.bfloat16, tag="xq", name="xq")
                nc.sync.dma_start(xq, qTp[:, :, nsl])
                for hp in range(NPAIR):
                    ps = psum.tile([128, SB], dt.float32, tag="mm", bufs=2,
                                   name="ps")
                    for t in range(NDT):
                        nc.tensor.matmul(
                            ps, wk_sb[:, t, hp * 128:(hp + 1) * 128],
                            xk[:, t, :],
                            start=(t == 0), stop=(t == NDT - 1))
                    nc.vector.tensor_scalar_add(
                        kt_sb[:, hp, nsl], ps, bk_sb[:, hp:hp + 1])
                for sv in range(SB // 128):
                    st_i = nb * (SB // 128) + sv
                    ps = psum.tile([128, SB], dt.float32, tag="mm", bufs=2,
                                   name="ps")[:, 0:GD]
                    for t in range(NDT):
                        nc.tensor.matmul(
                            ps, xv[:, t, sv * 128:(sv + 1) * 128],
                            wv_sb[:, t, :],
                            start=(t == 0), stop=(t == NDT - 1))
                    nc.vector.tensor_copy(
                        va_sb[:, st_i, :, 0:DK],
                        ps.rearrange("p (h d) -> p h d", h=4))
                for hp in range(NPAIR):
                    ps = psum.tile([128, SB], dt.float32, tag="mm", bufs=2,
                                   name="ps")
                    for t in range(NDT):
                        nc.tensor.matmul(
                            ps, wq_sb[:, t, hp * 128:(hp + 1) * 128],
                            xq[:, t, :],
                            start=(t == 0), stop=(t == NDT - 1))
                    nc.vector.tensor_scalar_add(
                        qt_sb[:, hp, nsl], ps, bq_sb[:, hp:hp + 1])

            # ---- output projection piece (2 PSUM banks + copies + 1 DMA
            # per 128-row s-tile); emitted interleaved into the following
            # q-block's attention so it fills PE slack in the ACT-bound
            # stream instead of serializing at the end -------------------
            def oproj_piece(st_i):
                ssl = slice(st_i * 128, (st_i + 1) * 128)
                ot = ostage.tile([128, D], dt.float32, tag="ot", name="ot")
                for nb2 in range(D // SB):
                    ps = psum.tile([128, SB], dt.float32, tag="mm", bufs=2,
                                   name="ps")
                    for hp in range(NPAIR):
                        nc.tensor.matmul(
                            ps, yt_sb[:, hp, ssl],
                            wo_sb[:, hp, nb2 * SB:(nb2 + 1) * SB],
                            start=(hp == 0), stop=(hp == NPAIR - 1))
                    nc.vector.tensor_copy(
                        ot[:, nb2 * SB:(nb2 + 1) * SB], ps)
                nc.sync.dma_start(out[ssl, :], ot)

            # ---- per q-block: attention ---------------------------------
            # scores^T [k, q] for both heads of the pair on disjoint PE
            # row-groups into one 2-bank PSUM tile; one wide exp (PSUM ->
            # SBUF bf16); PV accumulates yT_aug[65, q] with lhsT = [V_h | 1]
            # so row 64 = sum(exp). PV is emitted 2 k-tiles behind its exp
            # so the PE never sits at an et semaphore right after ACT.
            PVLAG = 2
            for qb in range(NSB):
                qsl = slice(qb * SB, (qb + 1) * SB)
                for hp in range(NPAIR):
                    ya = psum.tile([DK + 1, 2, SB], dt.float32, tag="ya",
                                   bufs=1, name="ya")

                    def pv(kk, et):
                        for j in range(2):
                            nc.tensor.matmul(
                                ya[:, j, :], va_sb[:, kk, 2 * hp + j, :],
                                et[:, j, :],
                                start=(kk == 0), stop=(kk == NKT - 1))

                    pend = []
                    for kk in range(NKT):
                        ksl = slice(kk * 128, (kk + 1) * 128)
                        st = psum.tile([128, 2, SB], dt.float32, tag="st",
                                       bufs=2, name="st")
                        nc.tensor.matmul(
                            st[:, 0, :],
                            kt_sb[0:DK, hp, ksl], qt_sb[0:DK, hp, qsl],
                            start=True, stop=True, tile_position=(0, 0))
                        nc.tensor.matmul(
                            st[:, 1, :],
                            kt_sb[DK:128, hp, ksl], qt_sb[DK:128, hp, qsl],
                            start=True, stop=True, tile_position=(64, 0))
                        et = estage.tile([128, 2, SB], dt.bfloat16, tag="et",
                                         name="et")
                        nc.scalar.activation(et[:], st[:], Exp)
                        pend.append((kk, et))
                        if len(pend) > PVLAG:
                            pv(*pend.pop(0))
                        if hp == 0 and qb > 0 and kk in (3, 7, 11):
                            oproj_piece((qb - 1) * 4 + kk // 4)
                    for item in pend:
                        pv(*item)

                    # Copy yT_aug out of PSUM immediately (frees the ya bank
                    # for the next pair-block), then normalize off the
                    # critical path: 1/Z = exp(-ln(Z)) on ScalarE (stays in
                    # the exp table set; DVE reciprocal is 8 cyc/elem on a
                    # single lane), partition-broadcast on GpSimd, multiply
                    # on VectorE.
                    ys = norm.tile([DK + 1, 2, SB], dt.float32, tag="ys",
                                   bufs=3)
                    nc.vector.tensor_copy(ys[:], ya[:])
                    lz = norm.tile([1, 2, SB], dt.float32, tag="lz")
                    nc.scalar.activation(lz[:], ys[DK:DK + 1, :, :], Log)
                    rr = norm.tile([1, 2, SB], dt.float32, tag="rr")
                    nc.scalar.activation(rr[:], lz[:], Exp, scale=-1.0)
                    for j in range(2):
                        rb = norm.tile([DK, SB], dt.float32, tag="rb")
                        nc.gpsimd.partition_broadcast(rb, rr[:, j, :])
                        nc.vector.tensor_mul(
                            yt_sb[j * DK:(j + 1) * DK, hp, qsl],
                            ys[0:DK, j, :], rb)
                if qb > 0:
                    oproj_piece((qb - 1) * 4 + 3)
            for sv in range(SB // 128):
                oproj_piece((NSB - 1) * 4 + sv)

    nc.compile()
    return nc


def _get_program():
    if "nc" not in _CACHE:
        _CACHE["nc"] = _build_program()
    return _CACHE["nc"]


def make_in_maps(q, k, v, w_q, b_q, w_k, b_k, w_v, b_v, w_o, b_o):
    import ml_dtypes
    bf16 = ml_dtypes.bfloat16
    scale = 1.0 / np.sqrt(np.float32(DK))

    wqT = np.ascontiguousarray(w_q.T * scale)
    wkT = np.ascontiguousarray(w_k.T)
    wvT = np.ascontiguousarray(w_v.T)
    woT = np.ascontiguousarray(w_o.T)

    in_maps = []
    for b in range(B):
        qT = np.ascontiguousarray(q[b].T).astype(bf16)
        kT = np.ascontiguousarray(k[b].T).astype(bf16)
        vT = np.ascontiguousarray(v[b].T).astype(bf16)
        for g in range(NGROUPS):
            sl = slice(g * GD, (g + 1) * GD)
            in_maps.append({
                "qT": qT, "kT": kT, "vT": vT,
                "wqT": np.ascontiguousarray(wqT[:, sl]).astype(bf16),
                "wkT": np.ascontiguousarray(wkT[:, sl]).astype(bf16),
                "wvT": np.ascontiguousarray(wvT[:, sl]).astype(bf16),
                "woT": np.ascontiguousarray(woT[sl, :]).astype(bf16),
                "bq": np.ascontiguousarray(
                    (b_q[sl] * scale).reshape(NPAIR, 128).T).astype(np.float32),
                "bk": np.ascontiguousarray(
                    b_k[sl].reshape(NPAIR, 128).T).astype(np.float32),
            })
    return in_maps


def gather(results, w_o, b_v, b_o):
    corr = (b_v.astype(np.float64) @ w_o.T.astype(np.float64)
            + b_o.astype(np.float64)).astype(np.float32)
    out = np.empty((B, S, D), np.float32)
    for b in range(B):
        acc = np.zeros((S, D), np.float64)
        for g in range(NGROUPS):
            acc += results[b * NGROUPS + g]["out"]
        out[b] = acc.astype(np.float32) + corr
    return out


def kernel(q, k, v, w_q, b_q, w_k, b_k, w_v, b_v, w_o, b_o):
    from concourse.bass_utils import run_bass_kernel_spmd

    nc = _get_program()
    in_maps = make_in_maps(q, k, v, w_q, b_q, w_k, b_k, w_v, b_v, w_o, b_o)
    res = run_bass_kernel_spmd(nc, in_maps, list(range(NCORES)))
    return gather(res.results, w_o, b_v, b_o)


# revision 13
# speedup vs baseline: 1.3170x; 1.0031x over previous
"""Multi-head attention (B=2, S=2048, D=1024, H=16) on 8 Trainium2 cores.

Sharding: core = (batch b in {0,1}) x (head-group g in {0..3}, 4 heads each).
Each core computes its 4 heads end-to-end (Q/K/V projections restricted to the
group's 256 dims, attention, and the row-slice of the output projection) and
returns a partial [S, D] output; the host sums the 4 group partials per batch.

Device-side layouts (per core):
  qT/kT/vT  [D, S]   input activations, transposed on host, bf16
  wqT/wkT/wvT [D, 256] weight column-slices (wqT pre-scaled by 1/sqrt(dk)), bf16
  woT       [256, D] w_o.T row-slice, bf16
  bq/bk     [128, 2] per-partition bias ((b/8 for q), laid out [p, head-pair])
  out       [S, D]   fp32 partial output

b_v and b_o are handled exactly on the host: softmax rows sum to 1, so the
V-bias contributes b_v @ w_o.T + b_o as a constant row vector.
"""

import numpy as np

B, S, D = 2, 2048, 1024
H, DK = 16, 64
NCORES = 8
NGROUPS = 4                  # head-groups; 4 heads = 256 dims per group
GD = (H // NGROUPS) * DK     # 256 dims per group
NPAIR = 2                    # head-pairs per group (2 heads = 128 dims each)
SB = 512                     # s-block (matmul free dim / PSUM bank)
NSB = S // SB                # 4 s-blocks
NKT = S // 128               # 16 k-tiles of 128
NDT = D // 128               # 8 contraction tiles for projections
KG = 2                       # k-tiles per exp group (exp width = KG*512)

_CACHE = {}


def _build_program():
    from concourse import bacc, tile
    import concourse.mybir as mybir

    # Route both Exp and Ln to the combined natural_log_exp_and_others
    # activation-table set: the default first-match policy picks different
    # sets for Exp and Ln, reloading ACT tables (~2.7us) at every softmax
    # normalization. Stripping them from the other sets (order preserved, so
    # set ids stay valid) forces one load for the whole kernel.
    if not getattr(bacc, "_mha_act_tables_patched", False):
        _orig_tables = bacc.get_activation_tables

        def _patched_tables(arch):
            t = _orig_tables(arch)
            exp_ln = {mybir.ActivationFunctionType.Exp,
                      mybir.ActivationFunctionType.Ln}
            if any(exp_ln <= funcs for funcs in t.values()):
                for name, funcs in t.items():
                    if not exp_ln <= funcs:
                        t[name] = funcs - exp_ln
            return t

        bacc.get_activation_tables = _patched_tables
        bacc._mha_act_tables_patched = True

    dt = mybir.dt
    nc = bacc.Bacc("TRN2", target_bir_lowering=False, debug=False,
                   num_devices=NCORES)

    qT = nc.dram_tensor("qT", [D, S], dt.bfloat16, kind="ExternalInput").ap()
    kT = nc.dram_tensor("kT", [D, S], dt.bfloat16, kind="ExternalInput").ap()
    vT = nc.dram_tensor("vT", [D, S], dt.bfloat16, kind="ExternalInput").ap()
    wqT = nc.dram_tensor("wqT", [D, GD], dt.bfloat16, kind="ExternalInput").ap()
    wkT = nc.dram_tensor("wkT", [D, GD], dt.bfloat16, kind="ExternalInput").ap()
    wvT = nc.dram_tensor("wvT", [D, GD], dt.bfloat16, kind="ExternalInput").ap()
    woT = nc.dram_tensor("woT", [GD, D], dt.bfloat16, kind="ExternalInput").ap()
    bq = nc.dram_tensor("bq", [128, NPAIR], dt.float32, kind="ExternalInput").ap()
    bk = nc.dram_tensor("bk", [128, NPAIR], dt.float32, kind="ExternalInput").ap()
    out = nc.dram_tensor("out", [S, D], dt.float32, kind="ExternalOutput").ap()

    qT_t = qT.rearrange("(t p) s -> t p s", p=128)   # [8, 128, S]
    kT_t = kT.rearrange("(t p) s -> t p s", p=128)
    vT_t = vT.rearrange("(t p) s -> t p s", p=128)
    wqT_t = wqT.rearrange("(t p) m -> t p m", p=128)  # [8, 128, GD]
    wkT_t = wkT.rearrange("(t p) m -> t p m", p=128)
    wvT_t = wvT.rearrange("(t p) m -> t p m", p=128)
    woT_t = woT.rearrange("(t p) m -> t p m", p=128)  # [2, 128, D]

    Exp = mybir.ActivationFunctionType.Exp
    Log = mybir.ActivationFunctionType.Ln

    with tile.TileContext(nc) as tc:
        with (
            tc.tile_pool(name="const", bufs=1) as const,
            tc.tile_pool(name="xin", bufs=2) as xin,
            tc.tile_pool(name="acts", bufs=1) as acts,
            tc.tile_pool(name="estage", bufs=8) as estage,
            tc.tile_pool(name="norm", bufs=4) as norm,
            tc.tile_pool(name="ostage", bufs=3) as ostage,
            tc.tile_pool(name="psum", bufs=1, space="PSUM") as psum,
        ):
            # ---- resident constants -------------------------------------
            wq_sb = const.tile([128, NDT, GD], dt.bfloat16, tag="wq")
            wk_sb = const.tile([128, NDT, GD], dt.bfloat16, tag="wk")
            wv_sb = const.tile([128, NDT, GD], dt.bfloat16, tag="wv")
            wo_sb = const.tile([128, NPAIR, D], dt.bfloat16, tag="wo")
            bq_sb = const.tile([128, NPAIR], dt.float32, tag="bq")
            bk_sb = const.tile([128, NPAIR], dt.float32, tag="bk")
            nc.sync.dma_start(wk_sb[:], wkT.rearrange("(t p) m -> p t m", p=128))
            nc.sync.dma_start(bk_sb[:], bk)
            nc.sync.dma_start(wv_sb[:], wvT.rearrange("(t p) m -> p t m", p=128))
            nc.sync.dma_start(wq_sb[:], wqT.rearrange("(t p) m -> p t m", p=128))
            nc.sync.dma_start(bq_sb[:], bq)
            nc.sync.dma_start(wo_sb[:], woT.rearrange("(t p) m -> p t m", p=128))

            # ---- activation tiles (whole-group residents) ---------------
            # QT/KT: [d-within-pair(128), pair, s]; V_aug: [k(128), k-tile,
            # head(4), dk+1] with col 64 = ones (softmax denominator trick).
            qt_sb = acts.tile([128, NPAIR, S], dt.bfloat16, tag="qt")
            kt_sb = acts.tile([128, NPAIR, S], dt.bfloat16, tag="kt")
            va_sb = acts.tile([128, NKT, 4, DK + 1], dt.bfloat16, tag="va")
            yt_sb = acts.tile([128, NPAIR, S], dt.bfloat16, tag="yt")

            nc.vector.memset(va_sb[:, :, :, DK:DK + 1], 1.0)

            # ---- K / V projections, interleaved per s-block -------------
            # K:  KT[d_g, s] = sum_t wkT[t].T @ kT[t]   (+ bk per-partition)
            # V:  V[s, d_g]  = sum_t vT[t][:, s-tile].T @ wvT[t]
            kTp = kT.rearrange("(t p) s -> p t s", p=128)   # [128, 8, S]
            vTp = vT.rearrange("(t p) s -> p t s", p=128)
            qTp = qT.rearrange("(t p) s -> p t s", p=128)
            def kproj_piece(nb):
                nsl = slice(nb * SB, (nb + 1) * SB)
                xk = xin.tile([128, NDT, SB], dt.bfloat16, tag="xk", name="xk")
                nc.sync.dma_start(xk, kTp[:, :, nsl])
                for hp in range(NPAIR):
                    ps = psum.tile([128, SB], dt.float32, tag="mm", bufs=2,
                                   name="ps")
                    for t in range(NDT):
                        nc.tensor.matmul(
                            ps, wk_sb[:, t, hp * 128:(hp + 1) * 128],
                            xk[:, t, :],
                            start=(t == 0), stop=(t == NDT - 1))
                    nc.vector.tensor_scalar_add(
                        kt_sb[:, hp, nsl], ps, bk_sb[:, hp:hp + 1])

            def vproj_piece(nb):
                nsl = slice(nb * SB, (nb + 1) * SB)
                xv = xin.tile([128, NDT, SB], dt.bfloat16, tag="xv", name="xv")
                nc.sync.dma_start(xv, vTp[:, :, nsl])
                for sv in range(SB // 128):
                    st_i = nb * (SB // 128) + sv
                    ps = psum.tile([128, SB], dt.float32, tag="mm", bufs=2,
                                   name="ps")[:, 0:GD]
                    for t in range(NDT):
                        nc.tensor.matmul(
                            ps, xv[:, t, sv * 128:(sv + 1) * 128],
                            wv_sb[:, t, :],
                            start=(t == 0), stop=(t == NDT - 1))
                    nc.vector.tensor_copy(
                        va_sb[:, st_i, :, 0:DK],
                        ps.rearrange("p (h d) -> p h d", h=4))

            def qproj_piece(nb):
                nsl = slice(nb * SB, (nb + 1) * SB)
                xq = xin.tile([128, NDT, SB], dt.bfloat16, tag="xq", name="xq")
                nc.sync.dma_start(xq, qTp[:, :, nsl])
                for hp in range(NPAIR):
                    ps = psum.tile([128, SB], dt.float32, tag="mm", bufs=2,
                                   name="ps")
                    for t in range(NDT):
                        nc.tensor.matmul(
                            ps, wq_sb[:, t, hp * 128:(hp + 1) * 128],
                            xq[:, t, :],
                            start=(t == 0), stop=(t == NDT - 1))
                    nc.vector.tensor_scalar_add(
                        qt_sb[:, hp, nsl], ps, bq_sb[:, hp:hp + 1])

            # nb=0 of everything upfront; the rest injected into qb0-hp0's
            # attention stream (kk-paced; in-order PE makes deadlines safe:
            # a piece emitted before scores kk is complete before them).
            kproj_piece(0)
            vproj_piece(0)
            qproj_piece(0)
            proj_inject = {1: [("k", 1)], 2: [("v", 1)], 3: [("k", 2)],
                           5: [("v", 2)], 7: [("k", 3)], 9: [("v", 3)],
                           11: [("q", 1)], 13: [("q", 2)], 15: [("q", 3)]}
            proj_fns = {"k": kproj_piece, "v": vproj_piece, "q": qproj_piece}

            # ---- output projection piece (2 PSUM banks + copies + 1 DMA
            # per 128-row s-tile); emitted interleaved into the following
            # q-block's attention so it fills PE slack in the ACT-bound
            # stream instead of serializing at the end -------------------
            def oproj_piece(st_i):
                ssl = slice(st_i * 128, (st_i + 1) * 128)
                ot = ostage.tile([128, D], dt.float32, tag="ot", name="ot")
                for nb2 in range(D // SB):
                    ps = psum.tile([128, SB], dt.float32, tag="mm", bufs=2,
                                   name="ps")
                    for hp in range(NPAIR):
                        nc.tensor.matmul(
                            ps, yt_sb[:, hp, ssl],
                            wo_sb[:, hp, nb2 * SB:(nb2 + 1) * SB],
                            start=(hp == 0), stop=(hp == NPAIR - 1))
                    nc.vector.tensor_copy(
                        ot[:, nb2 * SB:(nb2 + 1) * SB], ps)
                nc.sync.dma_start(out[ssl, :], ot)

            # ---- per q-block: attention ---------------------------------
            # scores^T [k, q] for both heads of the pair on disjoint PE
            # row-groups into one 2-bank PSUM tile; one wide exp (PSUM ->
            # SBUF bf16); PV accumulates yT_aug[65, q] with lhsT = [V_h | 1]
            # so row 64 = sum(exp). PV is emitted 2 k-tiles behind its exp
            # so the PE never sits at an et semaphore right after ACT.
            PVLAG = 2
            for qb in range(NSB):
                qsl = slice(qb * SB, (qb + 1) * SB)
                for hp in range(NPAIR):
                    ya = psum.tile([DK + 1, 2, SB], dt.float32, tag="ya",
                                   bufs=1, name="ya")

                    def pv(kk, et):
                        for j in range(2):
                            nc.tensor.matmul(
                                ya[:, j, :], va_sb[:, kk, 2 * hp + j, :],
                                et[:, j, :],
                                start=(kk == 0), stop=(kk == NKT - 1))

                    pend = []
                    for kk in range(NKT):
                        ksl = slice(kk * 128, (kk + 1) * 128)
                        st = psum.tile([128, 2, SB], dt.float32, tag="st",
                                       bufs=2, name="st")
                        nc.tensor.matmul(
                            st[:, 0, :],
                            kt_sb[0:DK, hp, ksl], qt_sb[0:DK, hp, qsl],
                            start=True, stop=True, tile_position=(0, 0))
                        nc.tensor.matmul(
                            st[:, 1, :],
                            kt_sb[DK:128, hp, ksl], qt_sb[DK:128, hp, qsl],
                            start=True, stop=True, tile_position=(64, 0))
                        et = estage.tile([128, 2, SB], dt.bfloat16, tag="et",
                                         name="et")
                        nc.scalar.activation(et[:], st[:], Exp)
                        pend.append((kk, et))
                        if len(pend) > PVLAG:
                            pv(*pend.pop(0))
                        if hp == 0 and qb > 0 and kk in (3, 7, 11):
                            oproj_piece((qb - 1) * 4 + kk // 4)
                        if hp == 0 and qb == 0:
                            for kind, nb in proj_inject.get(kk, ()):
                                proj_fns[kind](nb)
                    for item in pend:
                        pv(*item)

                    # Copy yT_aug out of PSUM immediately (frees the ya bank
                    # for the next pair-block), then normalize off the
                    # critical path: 1/Z = exp(-ln(Z)) on ScalarE (stays in
                    # the exp table set; DVE reciprocal is 8 cyc/elem on a
                    # single lane), partition-broadcast on GpSimd, multiply
                    # on VectorE.
                    ys = norm.tile([DK + 1, 2, SB], dt.float32, tag="ys",
                                   bufs=3)
                    nc.vector.tensor_copy(ys[:], ya[:])
                    lz = norm.tile([1, 2, SB], dt.float32, tag="lz")
                    nc.scalar.activation(lz[:], ys[DK:DK + 1, :, :], Log)
                    rr = norm.tile([1, 2, SB], dt.float32, tag="rr")
                    nc.scalar.activation(rr[:], lz[:], Exp, scale=-1.0)
                    for j in range(2):
                        rb = norm.tile([DK, SB], dt.float32, tag="rb")
                        nc.gpsimd.partition_broadcast(rb, rr[:, j, :])
                        nc.vector.tensor_mul(
                            yt_sb[j * DK:(j + 1) * DK, hp, qsl],
                            ys[0:DK, j, :], rb)
                if qb > 0:
                    oproj_piece((qb - 1) * 4 + 3)
            for sv in range(SB // 128):
                oproj_piece((NSB - 1) * 4 + sv)

    nc.compile()
    return nc


def _get_program():
    if "nc" not in _CACHE:
        _CACHE["nc"] = _build_program()
    return _CACHE["nc"]


def make_in_maps(q, k, v, w_q, b_q, w_k, b_k, w_v, b_v, w_o, b_o):
    import ml_dtypes
    bf16 = ml_dtypes.bfloat16
    scale = 1.0 / np.sqrt(np.float32(DK))

    wqT = np.ascontiguousarray(w_q.T * scale)
    wkT = np.ascontiguousarray(w_k.T)
    wvT = np.ascontiguousarray(w_v.T)
    woT = np.ascontiguousarray(w_o.T)

    in_maps = []
    for b in range(B):
        qT = np.ascontiguousarray(q[b].T).astype(bf16)
        kT = np.ascontiguousarray(k[b].T).astype(bf16)
        vT = np.ascontiguousarray(v[b].T).astype(bf16)
        for g in range(NGROUPS):
            sl = slice(g * GD, (g + 1) * GD)
            in_maps.append({
                "qT": qT, "kT": kT, "vT": vT,
                "wqT": np.ascontiguousarray(wqT[:, sl]).astype(bf16),
                "wkT": np.ascontiguousarray(wkT[:, sl]).astype(bf16),
                "wvT": np.ascontiguousarray(wvT[:, sl]).astype(bf16),
                "woT": np.ascontiguousarray(woT[sl, :]).astype(bf16),
                "bq": np.ascontiguousarray(
                    (b_q[sl] * scale).reshape(NPAIR, 128).T).astype(np.float32),
                "bk": np.ascontiguousarray(
                    b_k[sl].reshape(NPAIR, 128).T).astype(np.float32),
            })
    return in_maps


def gather(results, w_o, b_v, b_o):
    corr = (b_v.astype(np.float64) @ w_o.T.astype(np.float64)
            + b_o.astype(np.float64)).astype(np.float32)
    out = np.empty((B, S, D), np.float32)
    for b in range(B):
        acc = np.zeros((S, D), np.float64)
        for g in range(NGROUPS):
            acc += results[b * NGROUPS + g]["out"]
        out[b] = acc.astype(np.float32) + corr
    return out


def kernel(q, k, v, w_q, b_q, w_k, b_k, w_v, b_v, w_o, b_o):
    from concourse.bass_utils import run_bass_kernel_spmd

    nc = _get_program()
    in_maps = make_in_maps(q, k, v, w_q, b_q, w_k, b_k, w_v, b_v, w_o, b_o)
    res = run_bass_kernel_spmd(nc, in_maps, list(range(NCORES)))
    return gather(res.results, w_o, b_v, b_o)
